# revision 14
# baseline (speedup 1.0000x reference)
"""Trainium2 Bass kernel for nn_BiAttn_TFN_hg_2desc_Net (GNN message passing).

Strategy (8 NeuronCores, SPMD single program):
  - Nodes/graphs sharded by graph (64 graphs/core, contiguous node ranges since
    graph_id is sorted). Edges sharded by dst-owner core.
  - Phase 1 (replicated): t1 = feat @ gc1_W, computed fully on every core from
    a host-pretransposed featT (bf16). Replication avoids any collective here.
  - Phase 2 (L1 edge aggregation): edges bucketed host-side by
    (dst 128-block, table half); payload rows t1[src] fetched with dma_gather
    (int16 idx, 1024 rows/instruction); segment-sum by dst done with one-hot
    selector matmuls (selector built on-device: is_equal(iota, dstrel)) into
    PSUM; epilogue h1 = relu(agg*rdeg + b1) with host-precomputed 1/deg, then
    t2 = h1 @ gc2_W via a PE transpose.
  - Phase 3: one AllGather of the tight t2 (bf16) across the 8 cores.
  - Phase 4: restride t2 to 256B rows for gathering.
  - Phase 5 (L2 edge aggregation): same machinery, 20-wide payload, then
    graph-mean pooling via one-hot graph-selector matmuls (counts from host).
  - Head: bilinear attention + fusion outer-product + 3-layer MLP with
    BatchNorm, computed feature-major; BN batch stats via two tiny AllReduces.
"""

import sys

sys.path.insert(0, "/opt/trn_rl_repo")

import numpy as np
import ml_dtypes

import concourse.bass as bass
import concourse.bacc as bacc
import concourse.tile as tile
from concourse import mybir
from concourse import bass_utils
from concourse.library_config import mlp as _mlp_lib

bass_utils.upload_artifacts = lambda tmpdir: "local://skipped"

P = 128
TG = 8          # tiles per dma_gather (1024 indices)
NI = P * TG
BN_EPS = 1e-5

F32 = mybir.dt.float32
BF16 = mybir.dt.bfloat16
I16 = mybir.dt.int16

BF = ml_dtypes.bfloat16


# ----------------------------------------------------------------------------
# Host-side planning
# ----------------------------------------------------------------------------

def _wrap_idx(flat_idx):
    """[NI] int -> [128, NI//16] int16 in the dma_gather wrapped layout
    (idx i at [i % 16, i // 16], tiled x8 down the partitions)."""
    a = np.asarray(flat_idx, np.int16).reshape(-1, 16).T      # [16, NI/16]
    return np.tile(a, (8, 1))                                  # [128, NI/16]


def _bucket_plan(src_list, dstrel_list, half_of, idx_in_half, n_tiles):
    """Pad one (block, half) bucket's edges to n_tiles*128 and emit per-tile
    idx (int32 within half) and dstrel (f32) arrays."""
    e = len(src_list)
    tot = n_tiles * P
    idx = np.zeros(tot, np.int64)
    dr = np.full(tot, 255.0, np.float32)
    if e:
        idx[:e] = idx_in_half
        dr[:e] = dstrel_list
    return idx, dr


def plan(inputs, nc_cores, dims):
    """Host preprocessing. Returns (meta, per_core_inputs)."""
    N = dims["N"]; E = dims["E"]; B = dims["B"]
    DIM_IN = dims["DIM_IN"]; GC1 = dims["GC1"]; DG = dims["DG"]
    D2 = dims["D2"]; DH = dims["DH"]; H1 = dims["H1"]; H2 = dims["H2"]
    NC = nc_cores
    GPC = B // NC

    feat = np.asarray(inputs["feat"], np.float32)
    src = np.asarray(inputs["src"], np.int64)
    dst = np.asarray(inputs["dst"], np.int64)
    gid = np.asarray(inputs["graph_id"], np.int64)

    # --- core node/graph ranges (graph-aligned) ---
    bounds = np.searchsorted(gid, np.arange(0, B + 1, GPC))
    g_start, g_end = bounds[:-1], bounds[1:]
    nodes_c = g_end - g_start
    B_blk = int(np.ceil(nodes_c.max() / P))
    NPAD = B_blk * P
    TOTPAD = NC * NPAD
    # t1 table halves (rows of t1, global node ids, padded to 128)
    NT1ROWS = int(np.ceil(N / P)) * P
    HALF1 = (NT1ROWS // P // 2 + (NT1ROWS // P) % 2) * P
    assert HALF1 < 32768 and NT1ROWS - HALF1 < 32768
    HALF2 = TOTPAD // 2
    assert HALF2 % P == 0 and HALF2 < 32768

    # --- degrees / counts ---
    deg = np.bincount(dst, minlength=N).astype(np.float32)
    rdeg_full = 1.0 / np.maximum(deg, 1.0)
    cnt = np.bincount(gid, minlength=B).astype(np.float32)
    rcnt_full = (1.0 / np.maximum(cnt, 1.0)).astype(np.float32)

    # --- edge assignment ---
    core_of_dst = np.searchsorted(g_end - 1, dst)          # g_start <= dst < g_end
    # L2 gather index: position of src in the padded allgathered table
    core_of_src = np.searchsorted(g_end - 1, src)
    src_pad = core_of_src * NPAD + (src - g_start[core_of_src])

    per_core_edges = []
    T1 = np.zeros((B_blk, 2), np.int64)
    T2 = np.zeros((B_blk, 2), np.int64)
    for c in range(NC):
        m = core_of_dst == c
        e_src, e_dst, e_srcpad = src[m], dst[m], src_pad[m]
        drel = e_dst - g_start[c]
        blk = drel // P
        drel_in = (drel % P).astype(np.float32)
        h1b = (e_src >= HALF1).astype(np.int64)
        h2b = (e_srcpad >= HALF2).astype(np.int64)
        buckets = {}
        for b in range(B_blk):
            mb = blk == b
            for h in (0, 1):
                m1 = mb & (h1b == h)
                buckets[("L1", b, h)] = (e_src[m1] - h * HALF1, drel_in[m1])
                T1[b, h] = max(T1[b, h], int(np.ceil(m1.sum() / P)))
                m2 = mb & (h2b == h)
                buckets[("L2", b, h)] = (e_srcpad[m2] - h * HALF2, drel_in[m2])
                T2[b, h] = max(T2[b, h], int(np.ceil(m2.sum() / P)))
        per_core_edges.append(buckets)
    T1 = np.maximum(T1, 1)   # keep >= 1 tile so psum groups are never empty
    T2 = np.maximum(T2, 1)

    NT1 = int(T1.sum()); NT2 = int(T2.sum())
    NG1 = int(sum(-(-int(T1[b, h]) // TG) for b in range(B_blk) for h in (0, 1)))
    NG2 = int(sum(-(-int(T2[b, h]) // TG) for b in range(B_blk) for h in (0, 1)))

    import os
    meta = dict(
        PH=int(os.environ.get("K_PHASES", "9")),
        NC=NC, B=B, GPC=GPC, B_blk=B_blk, NPAD=NPAD, TOTPAD=TOTPAD,
        NT1ROWS=NT1ROWS, HALF1=HALF1, HALF2=HALF2,
        T1=T1.tolist(), T2=T2.tolist(), NT1=NT1, NT2=NT2, NG1=NG1, NG2=NG2,
        DIM_IN=DIM_IN, GC1=GC1, DG=DG, D2=D2, DH=DH, H1=H1, H2=H2,
    )

    # --- shared (replicated) tensors ---
    featT = np.zeros((DIM_IN, NT1ROWS), BF)
    featT[:, :N] = feat.T.astype(BF)
    w1 = np.asarray(inputs["gc1_W"], np.float32).astype(BF)          # [128,100]
    w2 = np.asarray(inputs["gc2_W"], np.float32).astype(BF)          # [100,20]
    iota = np.tile(np.arange(P, dtype=np.float32), (P, 1))
    ident = np.eye(P, dtype=np.float32)
    b1b = np.tile(np.asarray(inputs["gc1_b"], np.float32), (P, 1))   # [128,100]
    b2b = np.tile(np.asarray(inputs["gc2_b"], np.float32), (P, 1))   # [128,20]

    pg_W = np.asarray(inputs["pg_W"], np.float32); pg_b = np.asarray(inputs["pg_b"], np.float32)
    p2_W = np.asarray(inputs["p2_W"], np.float32); p2_b = np.asarray(inputs["p2_b"], np.float32)
    W2m = np.asarray(inputs["W2"], np.float32)
    w2eff = np.concatenate([pg_W, pg_b[None, :]], 0) @ W2m            # [21, 64]
    p2w_aug = np.concatenate([p2_W, p2_b[None, :]], 0)                # [201, 64]
    FD = (DG + 1) * (D2 + 1)
    FDP = -(-FD // P) * P
    fc1w = np.zeros((FDP, H1), np.float32)
    fc1w[:FD] = np.asarray(inputs["fc1_W"], np.float32)
    fc1b_r = np.asarray(inputs["fc1_b"], np.float32)
    fc2w = np.asarray(inputs["fc2_W"], np.float32)
    fc2b_r = np.asarray(inputs["fc2_b"], np.float32)
    fc3w = np.asarray(inputs["fc3_W"], np.float32)
    fc3b_r = np.asarray(inputs["fc3_b"], np.float32)
    bn1g = np.asarray(inputs["bn1_g"], np.float32)[:, None]
    bn1b = np.asarray(inputs["bn1_b"], np.float32)[:, None]
    bn2g = np.asarray(inputs["bn2_g"], np.float32)[:, None]
    bn2b = np.asarray(inputs["bn2_b"], np.float32)[:, None]
    # fc biases ride along as [H,1] columns added before BN.
    # BN(x+c) absorbs additive consts into the mean, but relu(out@fc3+b) does
    # not, so fc1_b/fc2_b only matter through BN: BN(x + b) == BN(x) exactly
    # (mean shifts by b). So fc1_b and fc2_b cancel entirely; fc3_b survives.
    meta["FDP"] = FDP
    desc2d = np.asarray(inputs["desc_2d"], np.float32)                # [B, 200]

    per_core = []
    for c in range(NC):
        buckets = per_core_edges[c]
        idx1 = np.zeros((max(NG1, 1), P, NI // 16), np.int16)
        dr1 = np.zeros((P, NT1), np.float32)
        idx2 = np.zeros((max(NG2, 1), P, NI // 16), np.int16)
        dr2 = np.zeros((P, NT2), np.float32)
        for (L, idx_arr, dr_arr, T) in (("L1", idx1, dr1, T1), ("L2", idx2, dr2, T2)):
            g_i = 0
            t_i = 0
            for b in range(B_blk):
                for h in (0, 1):
                    nt = int(T[b, h])
                    ii, dd = buckets[(L, b, h)]
                    iidx, ddr = _bucket_plan(ii, dd, h, ii, nt)
                    # per-tile dstrel columns (partition-major)
                    dr_arr[:, t_i:t_i + nt] = ddr.reshape(nt, P).T
                    t_i += nt
                    # gather groups of up to TG tiles (last group exact-sized)
                    for g0 in range(0, nt, TG):
                        gtiles = min(TG, nt - g0)
                        flat = iidx[g0 * P:(g0 + gtiles) * P]
                        idx_arr[g_i, :, : gtiles * P // 16] = _wrap_idx(flat)
                        g_i += 1
        nloc = int(nodes_c[c])
        rdeg = np.ones((B_blk * P,), np.float32)
        rdeg[:nloc] = rdeg_full[g_start[c]:g_end[c]]
        gidrel = np.full((B_blk * P,), 255.0, np.float32)
        gidrel[:nloc] = (gid[g_start[c]:g_end[c]] - c * GPC).astype(np.float32)
        rcnt = rcnt_full[c * GPC:(c + 1) * GPC][:, None]              # [GPC,1]
        d2c = desc2d[c * GPC:(c + 1) * GPC]                            # [GPC,200]
        d2T_aug = np.concatenate([d2c.T, np.ones((1, GPC), np.float32)], 0)  # [201,GPC]
        per_core.append({
            "featT": featT, "w1": w1, "w2": w2, "iota": iota, "ident": ident,
            "b1b": b1b, "b2b": b2b,
            "idx1": idx1, "dr1": dr1, "idx2": idx2, "dr2": dr2,
            "rdeg": rdeg.reshape(B_blk, P).T.copy(),      # [128, B_blk]
            "gidrel": gidrel.reshape(B_blk, P).T.copy(),  # [128, B_blk]
            "rcnt": rcnt, "d2gm": d2c, "d2T": d2T_aug,
            "w2eff": w2eff, "p2w": p2w_aug,
            "fc1w": fc1w, "fc2w": fc2w, "fc3w": fc3w,
            "fc3b": np.array([[float(fc3b_r[0])]], np.float32),
            "bn1g": bn1g, "bn1b": bn1b, "bn2g": bn2g, "bn2b": bn2b,
        })
    return meta, per_core


# ----------------------------------------------------------------------------
# Device program
# ----------------------------------------------------------------------------

def build(meta):
    NC = meta["NC"]; B = meta["B"]; GPC = meta["GPC"]; B_blk = meta["B_blk"]
    NPAD = meta["NPAD"]; TOTPAD = meta["TOTPAD"]
    NT1ROWS = meta["NT1ROWS"]; HALF1 = meta["HALF1"]; HALF2 = meta["HALF2"]
    T1 = meta["T1"]; T2 = meta["T2"]; NT1 = meta["NT1"]; NT2 = meta["NT2"]
    NG1 = meta["NG1"]; NG2 = meta["NG2"]
    DIM_IN = meta["DIM_IN"]; GC1 = meta["GC1"]; DG = meta["DG"]; D2 = meta["D2"]
    H1 = meta["H1"]; H2 = meta["H2"]; FDP = meta["FDP"]; PH = meta["PH"]
    EQ = mybir.AluOpType.is_equal
    MUL = mybir.AluOpType.mult
    ADD = mybir.AluOpType.add
    SUB = mybir.AluOpType.subtract
    AF = mybir.ActivationFunctionType

    nc = bacc.Bacc("TRN2", target_bir_lowering=False, debug=False, num_devices=NC)

    def din(name, shape, dt):
        return nc.dram_tensor(name, shape, dt, kind="ExternalInput").ap()

    featT_d = din("featT", [DIM_IN, NT1ROWS], BF16)
    w1_d = din("w1", [DIM_IN, GC1], BF16)
    w2_d = din("w2", [GC1, DG], BF16)
    iota_d = din("iota", [P, P], F32)
    ident_d = din("ident", [P, P], F32)
    b1b_d = din("b1b", [P, GC1], F32)
    b2b_d = din("b2b", [P, DG], F32)
    idx1_d = din("idx1", [max(NG1, 1), P, NI // 16], I16)
    dr1_d = din("dr1", [P, NT1], F32)
    idx2_d = din("idx2", [max(NG2, 1), P, NI // 16], I16)
    dr2_d = din("dr2", [P, NT2], F32)
    rdeg_d = din("rdeg", [P, B_blk], F32)
    gidrel_d = din("gidrel", [P, B_blk], F32)
    rcnt_d = din("rcnt", [GPC, 1], F32)
    d2gm_d = din("d2gm", [GPC, D2], F32)
    d2T_d = din("d2T", [D2 + 1, GPC], F32)
    w2eff_d = din("w2eff", [DG + 1, 64], F32)
    p2w_d = din("p2w", [D2 + 1, 64], F32)
    fc1w_d = din("fc1w", [FDP, H1], F32)
    fc2w_d = din("fc2w", [H1, H2], F32)
    fc3w_d = din("fc3w", [H2, 1], F32)
    fc3b_d = din("fc3b", [1, 1], F32)
    bn1g_d = din("bn1g", [H1, 1], F32)
    bn1b_d = din("bn1b", [H1, 1], F32)
    bn2g_d = din("bn2g", [H2, 1], F32)
    bn2b_d = din("bn2b", [H2, 1], F32)

    t1_d = nc.dram_tensor("t1tab", [NT1ROWS, P], BF16).ap()
    t2sh_d = nc.dram_tensor("t2shard", [NPAD, 32], BF16).ap()
    t2full_d = nc.dram_tensor("t2full", [TOTPAD, 32], BF16, addr_space="Shared").ap()
    t2pad_d = nc.dram_tensor("t2pad", [TOTPAD, P], BF16).ap()
    bn1i_d = nc.dram_tensor("bn1i", [H1, 2], F32).ap()
    bn1o_d = nc.dram_tensor("bn1o", [H1, 2], F32, addr_space="Shared").ap()
    bn2i_d = nc.dram_tensor("bn2i", [H2, 2], F32).ap()
    bn2o_d = nc.dram_tensor("bn2o", [H2, 2], F32, addr_space="Shared").ap()
    out_d = nc.dram_tensor("out", [1, GPC], F32, kind="ExternalOutput").ap()

    groups = [list(range(NC))]

    class _SkipRest(Exception):
        pass

    with tile.TileContext(nc) as tc:
        from contextlib import ExitStack
        with ExitStack() as ctx:
          try:
            cp = ctx.enter_context(tc.tile_pool(name="consts", bufs=1))
            fpool = ctx.enter_context(tc.tile_pool(name="feat", bufs=3))
            pp_t1 = ctx.enter_context(tc.tile_pool(name="p_t1", bufs=2, space="PSUM"))
            sb_t1 = ctx.enter_context(tc.tile_pool(name="sb_t1", bufs=4))
            ip = ctx.enter_context(tc.tile_pool(name="idx", bufs=6))
            payp = ctx.enter_context(tc.tile_pool(name="pay", bufs=5))
            selp = ctx.enter_context(tc.tile_pool(name="sel", bufs=8))
            drp = ctx.enter_context(tc.tile_pool(name="dr", bufs=3))
            pp_agg = ctx.enter_context(tc.tile_pool(name="p_agg", bufs=2, space="PSUM"))
            pp_tr = ctx.enter_context(tc.tile_pool(name="p_tr", bufs=1, space="PSUM"))
            pp_t2 = ctx.enter_context(tc.tile_pool(name="p_t2", bufs=1, space="PSUM"))
            hpool = ctx.enter_context(tc.tile_pool(name="hwork", bufs=3))
            pp_hg = ctx.enter_context(tc.tile_pool(name="p_hg", bufs=1, space="PSUM"))
            hd = ctx.enter_context(tc.tile_pool(name="head", bufs=1))

            nc.gpsimd.load_library(_mlp_lib)

            # ---- constants ----
            iota_t = cp.tile([P, P], F32); nc.sync.dma_start(iota_t[:], iota_d[:])
            zcol = cp.tile([P, 1], F32); nc.vector.memset(zcol[:], 0.0)
            ident_t = cp.tile([P, P], F32); nc.sync.dma_start(ident_t[:], ident_d[:])
            w1_t = cp.tile([DIM_IN, GC1], BF16); nc.sync.dma_start(w1_t[:], w1_d[:])
            w2_t = cp.tile([GC1, DG], BF16); nc.sync.dma_start(w2_t[:], w2_d[:])
            b1b_t = cp.tile([P, GC1], F32); nc.sync.dma_start(b1b_t[:], b1b_d[:])
            b2b_t = cp.tile([P, DG], F32); nc.sync.dma_start(b2b_t[:], b2b_d[:])
            rdeg_t = cp.tile([P, B_blk], F32); nc.sync.dma_start(rdeg_t[:], rdeg_d[:])
            gidr_t = cp.tile([P, B_blk], F32); nc.sync.dma_start(gidr_t[:], gidrel_d[:])

            # ================= Phase 1: t1 = feat @ W1 (replicated) ==========
            n_nt = NT1ROWS // P
            CHUNK = 16
            for c0 in range(0, n_nt, CHUNK):
                cw = min(CHUNK, n_nt - c0)
                ft = fpool.tile([P, CHUNK * P], BF16, tag="featT")
                nc.sync.dma_start(ft[:, :cw * P], featT_d[:, c0 * P:(c0 + cw) * P])
                for t in range(cw):
                    ps = pp_t1.tile([P, GC1], F32, tag="t1ps")
                    nc.tensor.matmul(ps[:], lhsT=ft[:, t * P:(t + 1) * P],
                                     rhs=w1_t[:], start=True, stop=True)
                    sb = sb_t1.tile([P, P], BF16, tag="t1sb")
                    nc.vector.tensor_copy(sb[:, :GC1], ps[:])
                    nc.vector.memset(sb[:, GC1:], 0.0)
                    nt = c0 + t
                    nc.sync.dma_start(t1_d[nt * P:(nt + 1) * P, :], sb[:])

            # ================= Phase 2/5 shared edge-layer builder ===========
            def edge_layer(layer, tab_d, half, T, ng_base_unused, idx_dram, dr_dram,
                           ncols, bias_t, out_block):
                """Per dst-block: gather + one-hot matmul segsum + epilogue."""
                g_i = 0
                t_i = 0
                for b in range(B_blk):
                    Tb = int(T[b][0]) + int(T[b][1])
                    dr_sb = drp.tile([P, max(Tb, 1)], F32, tag="drsb")
                    nc.sync.dma_start(dr_sb[:, :Tb], dr_dram[:, t_i:t_i + Tb])
                    ps = pp_agg.tile([P, ncols], F32, tag="agg")
                    k = 0
                    for h in (0, 1):
                        nt = int(T[b][h])
                        for g0 in range(0, nt, TG):
                            gt = min(TG, nt - g0)
                            ni = gt * P
                            ix = ip.tile([P, NI // 16], I16, tag="ix")
                            nc.sync.dma_start(ix[:, :ni // 16], idx_dram[g_i, :, :ni // 16])
                            pay = payp.tile([P, TG, P], BF16, tag="pay")
                            hi = min((h + 1) * half, tab_d.shape[0])
                            nc.gpsimd.dma_gather(
                                pay[:, :gt, :], tab_d[h * half:hi, :], ix[:, :ni // 16],
                                ni, ni, P)
                            for cc in range(gt):
                                sel = selp.tile([P, P], BF16, tag="sel")
                                nc.vector.tensor_scalar(
                                    out=sel[:], in0=iota_t[:],
                                    scalar1=dr_sb[:, k:k + 1], scalar2=None,
                                    op0=EQ)
                                nc.tensor.matmul(
                                    ps[:], lhsT=sel[:], rhs=pay[:, cc, :ncols],
                                    start=(k == 0), stop=(k == Tb - 1))
                                k += 1
                            g_i += 1
                    t_i += Tb
                    out_block(b, ps)

            # ---- Phase 2: layer 1 ----
            def l1_out(b, ps):
                h1 = hpool.tile([P, GC1], F32, tag="h1")
                nc.vector.tensor_scalar(out=h1[:], in0=ps[:],
                                        scalar1=rdeg_t[:, b:b + 1], scalar2=None,
                                        op0=MUL)
                nc.vector.tensor_tensor(out=h1[:], in0=h1[:], in1=b1b_t[:], op=ADD)
                nc.scalar.activation(out=h1[:], in_=h1[:], func=AF.Relu, bias=zcol[:, :1])
                tp = pp_tr.tile([GC1, P], F32, tag="trp")
                nc.tensor.transpose(tp[:], h1[:], ident_t[:])
                h1T = hpool.tile([GC1, P], BF16, tag="h1T")
                nc.vector.tensor_copy(h1T[:], tp[:])
                t2p = pp_t2.tile([P, DG], F32, tag="t2p")
                nc.tensor.matmul(t2p[:], lhsT=h1T[:], rhs=w2_t[:], start=True, stop=True)
                t2s = sb_t1.tile([P, P], BF16, tag="t2s")
                nc.vector.tensor_copy(t2s[:, :DG], t2p[:])
                nc.vector.memset(t2s[:, DG:], 0.0)
                nc.sync.dma_start(t2sh_d[b * P:(b + 1) * P, :], t2s[:, :32])

            if PH >= 2:
                edge_layer("1", t1_d, HALF1, T1, 0, idx1_d, dr1_d, GC1, b1b_t, l1_out)

            # ---- Phase 3: AllGather t2 ----
            if PH >= 3:
              nc.gpsimd.collective_compute(
                "AllGather", mybir.AluOpType.bypass, replica_groups=groups,
                ins=[t2sh_d[:].opt()], outs=[t2full_d[:].opt()])

            # ---- Phase 4: restride tight [*,32] -> padded [*,128] ----
            for i in range(TOTPAD // P if PH >= 4 else 0):
                rs = sb_t1.tile([P, P], BF16, tag="rs")
                nc.sync.dma_start(rs[:, :32], t2full_d[i * P:(i + 1) * P, :])
                nc.vector.memset(rs[:, 32:], 0.0)
                nc.sync.dma_start(t2pad_d[i * P:(i + 1) * P, :], rs[:])

            # ---- Phase 5: layer 2 + pooling ----
            do_l2 = PH >= 5
            do_head = PH >= 6
            hg_ps = pp_hg.tile([GPC, DG], F32, tag="hgps")

            def l2_out(b, ps):
                h2t = hpool.tile([P, DG], F32, tag="h2")
                nc.vector.tensor_scalar(out=h2t[:], in0=ps[:],
                                        scalar1=rdeg_t[:, b:b + 1], scalar2=None,
                                        op0=MUL)
                nc.vector.tensor_tensor(out=h2t[:], in0=h2t[:], in1=b2b_t[:], op=ADD)
                nc.scalar.activation(out=h2t[:], in_=h2t[:], func=AF.Relu, bias=zcol[:P, :1])
                selg = selp.tile([P, GPC], F32, tag="selg")
                nc.vector.tensor_scalar(out=selg[:], in0=iota_t[:, :GPC],
                                        scalar1=gidr_t[:, b:b + 1], scalar2=None,
                                        op0=EQ)
                nc.tensor.matmul(hg_ps[:], lhsT=selg[:], rhs=h2t[:],
                                 start=(b == 0), stop=(b == B_blk - 1),
                                 skip_group_check=True)

            if do_l2:
                edge_layer("2", t2pad_d, HALF2, T2, NG1, idx2_d, dr2_d, DG, b2b_t, l2_out)

            # ================= Head ==========================================
            if not do_head:
                raise _SkipRest()
            rcnt_t = hd.tile([GPC, 1], F32); nc.sync.dma_start(rcnt_t[:], rcnt_d[:])
            d2gm_t = hd.tile([GPC, D2], F32); nc.sync.dma_start(d2gm_t[:], d2gm_d[:])
            d2T_a = hd.tile([P, GPC], F32); nc.sync.dma_start(d2T_a[:], d2T_d[:P, :])
            d2T_b = hd.tile([D2 + 1 - P, GPC], F32); nc.sync.dma_start(d2T_b[:], d2T_d[P:, :])
            w2e_t = hd.tile([DG + 1, 64], F32); nc.sync.dma_start(w2e_t[:], w2eff_d[:])
            p2w_a = hd.tile([P, 64], F32); nc.sync.dma_start(p2w_a[:], p2w_d[:P, :])
            p2w_b = hd.tile([D2 + 1 - P, 64], F32); nc.sync.dma_start(p2w_b[:], p2w_d[P:, :])
            fc1w_t = hd.tile([P, FDP // P, H1], F32)
            nc.sync.dma_start(fc1w_t[:], fc1w_d[:].rearrange("(c p) h -> p c h", p=P))
            fc2w_t = hd.tile([H1, H2], F32); nc.sync.dma_start(fc2w_t[:], fc2w_d[:])
            fc3w_t = hd.tile([H2, 1], F32); nc.sync.dma_start(fc3w_t[:], fc3w_d[:])
            fc3b_t = hd.tile([1, 1], F32); nc.sync.dma_start(fc3b_t[:], fc3b_d[:])
            bn1g_t = hd.tile([H1, 1], F32); nc.sync.dma_start(bn1g_t[:], bn1g_d[:])
            bn1b_t = hd.tile([H1, 1], F32); nc.sync.dma_start(bn1b_t[:], bn1b_d[:])
            bn2g_t = hd.tile([H2, 1], F32); nc.sync.dma_start(bn2g_t[:], bn2g_d[:])
            bn2b_t = hd.tile([H2, 1], F32); nc.sync.dma_start(bn2b_t[:], bn2b_d[:])
            if PH == 60:
                raise _SkipRest()

            # hg1 = [hg | 1]
            hg1 = hd.tile([GPC, DG + 1], F32)
            nc.vector.tensor_scalar(out=hg1[:, :DG], in0=hg_ps[:], scalar1=rcnt_t[:, :1],
                                    scalar2=None, op0=MUL)
            nc.vector.memset(hg1[:, DG:DG + 1], 1.0)
            # hgT
            tp2 = pp_tr.tile([DG + 1, GPC], F32, tag="trp")
            nc.tensor.transpose(tp2[:], hg1[:], ident_t[:GPC, :GPC])
            hgT = hd.tile([DG + 1, GPC], F32)
            nc.vector.tensor_copy(hgT[:], tp2[:])
            # h_gm, h_d (graph-major [GPC, 64])
            hgm_ps = pp_t1.tile([GPC, 64], F32, tag="t1ps")
            nc.tensor.matmul(hgm_ps[:], lhsT=hgT[:], rhs=w2e_t[:], start=True, stop=True)
            hdm_ps = pp_t1.tile([GPC, 64], F32, tag="t1ps")
            nc.tensor.matmul(hdm_ps[:], lhsT=d2T_a[:], rhs=p2w_a[:],
                             start=True, stop=False)
            nc.tensor.matmul(hdm_ps[:], lhsT=d2T_b[:], rhs=p2w_b[:],
                             start=False, stop=True)
            hgm_sb = hd.tile([GPC, 64], F32)
            nc.vector.tensor_copy(hgm_sb[:], hgm_ps[:])
            junk = hd.tile([GPC, 64], F32)
            s_t = hd.tile([GPC, 1], F32)
            nc.vector.tensor_tensor(out=junk[:], in0=hgm_sb[:], in1=hdm_ps[:], op=MUL)
            nc.vector.reduce_sum(out=s_t[:], in_=junk[:], axis=mybir.AxisListType.X)
            a_t = hd.tile([GPC, 1], F32)
            nc.scalar.activation(out=a_t[:], in_=s_t[:], func=AF.Sigmoid, bias=zcol[:GPC, :1])
            if PH == 61:
                raise _SkipRest()
            # d1 = [a * desc2d | 1]
            d1 = hd.tile([GPC, D2 + 1], F32)
            nc.vector.tensor_scalar(out=d1[:, :D2], in0=d2gm_t[:], scalar1=a_t[:, :1],
                                    scalar2=None, op0=MUL)
            nc.vector.memset(d1[:, D2:D2 + 1], 1.0)
            # fusion [GPC, FDP]
            fus = hd.tile([GPC, FDP], F32)
            for i in range(DG + 1):
                nc.vector.tensor_scalar(out=fus[:, i * (D2 + 1):(i + 1) * (D2 + 1)],
                                        in0=d1[:], scalar1=hg1[:, i:i + 1],
                                        scalar2=None, op0=MUL)
            FD = (DG + 1) * (D2 + 1)
            if FDP > FD:
                nc.vector.memset(fus[:, FD:], 0.0)
            if PH == 62:
                raise _SkipRest()
            # fc1 (feature-major out [H1, GPC])
            fc1_ps = pp_t1.tile([H1, GPC], F32, tag="t1ps")
            for kt in range(FDP // P):
                ftp = pp_tr.tile([P, GPC], F32, tag="trp")
                nc.tensor.transpose(ftp[:], fus[:, kt * P:(kt + 1) * P],
                                    ident_t[:GPC, :GPC])
                fT = hpool.tile([P, GPC], F32, tag="fT")
                nc.vector.tensor_copy(fT[:], ftp[:])
                nc.tensor.matmul(fc1_ps[:], lhsT=fc1w_t[:, kt, :], rhs=fT[:],
                                 start=(kt == 0), stop=(kt == FDP // P - 1),
                                 skip_group_check=True)

            if PH < 7:
                raise _SkipRest()

            def bn_relu(x_ps, Hdim, g_t, b_t, bni_d, bno_d, tagp):
                xsb = hd.tile([Hdim, GPC], F32, name=f"xsb{tagp}")
                nc.vector.tensor_copy(xsb[:], x_ps[:])
                sums = hd.tile([Hdim, 1], F32, name=f"sums{tagp}")
                nc.vector.reduce_sum(out=sums[:], in_=xsb[:], axis=mybir.AxisListType.X)
                sqj = hd.tile([Hdim, GPC], F32, name=f"sqj{tagp}")
                sumsq = hd.tile([Hdim, 1], F32, name=f"sumsq{tagp}")
                nc.vector.tensor_tensor(out=sqj[:], in0=xsb[:], in1=xsb[:], op=MUL)
                nc.vector.reduce_sum(out=sumsq[:], in_=sqj[:], axis=mybir.AxisListType.X)
                stat = hd.tile([Hdim, 2], F32, name=f"stat{tagp}")
                nc.vector.tensor_copy(stat[:, 0:1], sums[:])
                nc.vector.tensor_copy(stat[:, 1:2], sumsq[:])
                nc.sync.dma_start(bni_d[:], stat[:])
                nc.gpsimd.collective_compute(
                    "AllReduce", ADD, replica_groups=groups,
                    ins=[bni_d[:].opt()], outs=[bno_d[:].opt()])
                statg = hd.tile([Hdim, 2], F32, name=f"statg{tagp}")
                nc.sync.dma_start(statg[:], bno_d[:])
                mean = hd.tile([Hdim, 1], F32, name=f"mean{tagp}")
                nc.vector.tensor_scalar(out=mean[:], in0=statg[:, 0:1],
                                        scalar1=1.0 / B, scalar2=None, op0=MUL)
                var = hd.tile([Hdim, 1], F32, name=f"var{tagp}")
                nc.vector.tensor_scalar(out=var[:], in0=statg[:, 1:2],
                                        scalar1=1.0 / B, scalar2=None, op0=MUL)
                msq = hd.tile([Hdim, 1], F32, name=f"msq{tagp}")
                nc.vector.tensor_tensor(out=msq[:], in0=mean[:], in1=mean[:], op=MUL)
                nc.vector.tensor_tensor(out=var[:], in0=var[:], in1=msq[:], op=SUB)
                nc.vector.tensor_scalar(out=var[:], in0=var[:], scalar1=BN_EPS,
                                        scalar2=None, op0=ADD)
                sd = hd.tile([Hdim, 1], F32, name=f"sd{tagp}")
                nc.scalar.activation(out=sd[:], in_=var[:], func=AF.Sqrt, bias=zcol[:Hdim, :1])
                rsd = hd.tile([Hdim, 1], F32, name=f"rsd{tagp}")
                nc.vector.reciprocal(rsd[:], sd[:])
                scl = hd.tile([Hdim, 1], F32, name=f"scl{tagp}")
                nc.vector.tensor_tensor(out=scl[:], in0=rsd[:], in1=g_t[:], op=MUL)
                tb = hd.tile([Hdim, 1], F32, name=f"tb{tagp}")
                nc.vector.tensor_tensor(out=tb[:], in0=mean[:], in1=scl[:], op=MUL)
                nc.vector.tensor_scalar(out=tb[:], in0=tb[:], scalar1=-1.0,
                                        scalar2=None, op0=MUL)
                nc.vector.tensor_tensor(out=tb[:], in0=tb[:], in1=b_t[:], op=ADD)
                o = hd.tile([Hdim, GPC], F32, name=f"bno{tagp}")
                nc.scalar.activation(out=o[:], in_=xsb[:], func=AF.Relu,
                                     bias=tb[:, 0:1], scale=scl[:, 0:1])
                return o

            bn1o_t = bn_relu(fc1_ps, H1, bn1g_t, bn1b_t, bn1i_d, bn1o_d, "1")
            if PH < 8:
                raise _SkipRest()
            fc2_ps = pp_t1.tile([H2, GPC], F32, tag="t1ps")
            nc.tensor.matmul(fc2_ps[:], lhsT=fc2w_t[:], rhs=bn1o_t[:], start=True, stop=True)
            bn2o_t = bn_relu(fc2_ps, H2, bn2g_t, bn2b_t, bn2i_d, bn2o_d, "2")
            fc3_ps = pp_t1.tile([1, GPC], F32, tag="t1ps")
            nc.tensor.matmul(fc3_ps[:], lhsT=fc3w_t[:], rhs=bn2o_t[:], start=True, stop=True)
            outsb = hd.tile([1, GPC], F32)
            nc.vector.tensor_scalar(out=outsb[:], in0=fc3_ps[:],
                                    scalar1=fc3b_t[0:1, 0:1], scalar2=None, op0=ADD)
            nc.sync.dma_start(out_d[:], outsb[:])
          except _SkipRest:
            pass

    nc.compile()
    return nc


# ----------------------------------------------------------------------------
# Entry point
# ----------------------------------------------------------------------------

REAL_DIMS = dict(N=50000, E=800000, B=512, DIM_IN=128, GC1=100, DG=20,
                 D2=200, DH=64, H1=128, H2=32)
_CACHE = {}


def run(inputs, nc_cores=8, dims=None, trace=False):
    dims = dims or REAL_DIMS
    meta, per_core = plan(inputs, nc_cores, dims)
    key = repr(sorted(meta.items()))
    if key not in _CACHE:
        _CACHE[key] = build(meta)
    prog = _CACHE[key]
    from concourse.bass_utils import run_bass_kernel_spmd
    res = run_bass_kernel_spmd(prog, per_core, list(range(nc_cores)), trace=trace)
    outs = [np.asarray(res.results[c]["out"]).reshape(-1) for c in range(nc_cores)]
    y = np.concatenate(outs).astype(np.float32)[:, None]
    return y, res


def kernel(**inputs):
    y, _ = run(inputs, nc_cores=8, dims=REAL_DIMS, trace=False)
    return y


# revision 16
# speedup vs baseline: 1.0218x; 1.0218x over previous
"""Trainium2 Bass kernel for nn_BiAttn_TFN_hg_2desc_Net (GNN message passing).

Strategy (8 NeuronCores, SPMD single program):
  - Nodes/graphs sharded by graph (64 graphs/core, contiguous node ranges since
    graph_id is sorted). Edges sharded by dst-owner core.
  - Phase 1 (replicated): t1 = feat @ gc1_W, computed fully on every core from
    a host-pretransposed featT (bf16). Replication avoids any collective here.
  - Phase 2 (L1 edge aggregation): edges bucketed host-side by
    (dst 128-block, table half); payload rows t1[src] fetched with dma_gather
    (int16 idx, 1024 rows/instruction); segment-sum by dst done with one-hot
    selector matmuls (selector built on-device: is_equal(iota, dstrel)) into
    PSUM; epilogue h1 = relu(agg*rdeg + b1) with host-precomputed 1/deg, then
    t2 = h1 @ gc2_W via a PE transpose.
  - Phase 3: one AllGather of the tight t2 (bf16) across the 8 cores.
  - Phase 4: restride t2 to 256B rows for gathering.
  - Phase 5 (L2 edge aggregation): same machinery, 20-wide payload, then
    graph-mean pooling via one-hot graph-selector matmuls (counts from host).
  - Head: bilinear attention + fusion outer-product + 3-layer MLP with
    BatchNorm, computed feature-major; BN batch stats via two tiny AllReduces.
"""

import sys

sys.path.insert(0, "/opt/trn_rl_repo")

import numpy as np
import ml_dtypes

import concourse.bass as bass
import concourse.bacc as bacc
import concourse.tile as tile
from concourse import mybir
from concourse import bass_utils
from concourse.library_config import mlp as _mlp_lib

bass_utils.upload_artifacts = lambda tmpdir: "local://skipped"

P = 128
TG = 8          # tiles per dma_gather (1024 indices)
NI = P * TG
BN_EPS = 1e-5

F32 = mybir.dt.float32
BF16 = mybir.dt.bfloat16
I16 = mybir.dt.int16

BF = ml_dtypes.bfloat16


# ----------------------------------------------------------------------------
# Host-side planning
# ----------------------------------------------------------------------------

def _wrap_idx(flat_idx):
    """[NI] int -> [128, NI//16] int16 in the dma_gather wrapped layout
    (idx i at [i % 16, i // 16], tiled x8 down the partitions)."""
    a = np.asarray(flat_idx, np.int16).reshape(-1, 16).T      # [16, NI/16]
    return np.tile(a, (8, 1))                                  # [128, NI/16]


def _bucket_plan(src_list, dstrel_list, half_of, idx_in_half, n_tiles):
    """Pad one (block, half) bucket's edges to n_tiles*128 and emit per-tile
    idx (int32 within half) and dstrel (f32) arrays."""
    e = len(src_list)
    tot = n_tiles * P
    idx = np.zeros(tot, np.int64)
    dr = np.full(tot, 255.0, np.float32)
    if e:
        idx[:e] = idx_in_half
        dr[:e] = dstrel_list
    return idx, dr


def plan(inputs, nc_cores, dims):
    """Host preprocessing. Returns (meta, per_core_inputs)."""
    N = dims["N"]; E = dims["E"]; B = dims["B"]
    DIM_IN = dims["DIM_IN"]; GC1 = dims["GC1"]; DG = dims["DG"]
    D2 = dims["D2"]; DH = dims["DH"]; H1 = dims["H1"]; H2 = dims["H2"]
    NC = nc_cores
    GPC = B // NC

    feat = np.asarray(inputs["feat"], np.float32)
    src = np.asarray(inputs["src"], np.int64)
    dst = np.asarray(inputs["dst"], np.int64)
    gid = np.asarray(inputs["graph_id"], np.int64)

    # --- core node/graph ranges (graph-aligned) ---
    bounds = np.searchsorted(gid, np.arange(0, B + 1, GPC))
    g_start, g_end = bounds[:-1], bounds[1:]
    nodes_c = g_end - g_start
    B_blk = int(np.ceil(nodes_c.max() / P))
    NPAD = B_blk * P
    TOTPAD = NC * NPAD
    # t1 table halves (rows of t1, global node ids, padded to 128)
    NT1ROWS = int(np.ceil(N / P)) * P
    HALF1 = (NT1ROWS // P // 2 + (NT1ROWS // P) % 2) * P
    assert HALF1 < 32768 and NT1ROWS - HALF1 < 32768
    HALF2 = TOTPAD // 2
    assert HALF2 % P == 0 and HALF2 < 32768

    # --- degrees / counts ---
    deg = np.bincount(dst, minlength=N).astype(np.float32)
    rdeg_full = 1.0 / np.maximum(deg, 1.0)
    cnt = np.bincount(gid, minlength=B).astype(np.float32)
    rcnt_full = (1.0 / np.maximum(cnt, 1.0)).astype(np.float32)

    # --- edge assignment ---
    core_of_dst = np.searchsorted(g_end - 1, dst)          # g_start <= dst < g_end
    # L2 gather index: position of src in the padded allgathered table
    core_of_src = np.searchsorted(g_end - 1, src)
    src_pad = core_of_src * NPAD + (src - g_start[core_of_src])

    per_core_edges = []
    T1 = np.zeros((B_blk, 2), np.int64)
    T2 = np.zeros((B_blk, 2), np.int64)
    for c in range(NC):
        m = core_of_dst == c
        e_src, e_dst, e_srcpad = src[m], dst[m], src_pad[m]
        drel = e_dst - g_start[c]
        blk = drel // P
        drel_in = (drel % P).astype(np.float32)
        h1b = (e_src >= HALF1).astype(np.int64)
        h2b = (e_srcpad >= HALF2).astype(np.int64)
        buckets = {}
        for b in range(B_blk):
            mb = blk == b
            for h in (0, 1):
                m1 = mb & (h1b == h)
                buckets[("L1", b, h)] = (e_src[m1] - h * HALF1, drel_in[m1])
                T1[b, h] = max(T1[b, h], int(np.ceil(m1.sum() / P)))
                m2 = mb & (h2b == h)
                buckets[("L2", b, h)] = (e_srcpad[m2] - h * HALF2, drel_in[m2])
                T2[b, h] = max(T2[b, h], int(np.ceil(m2.sum() / P)))
        per_core_edges.append(buckets)
    T1 = np.maximum(T1, 1)   # keep >= 1 tile so psum groups are never empty
    T2 = np.maximum(T2, 1)

    NT1 = int(T1.sum()); NT2 = int(T2.sum())
    NG1 = int(sum(-(-int(T1[b, h]) // TG) for b in range(B_blk) for h in (0, 1)))
    NG2 = int(sum(-(-int(T2[b, h]) // TG) for b in range(B_blk) for h in (0, 1)))

    import os
    meta = dict(
        PH=int(os.environ.get("K_PHASES", "9")),
        NC=NC, B=B, GPC=GPC, B_blk=B_blk, NPAD=NPAD, TOTPAD=TOTPAD,
        NT1ROWS=NT1ROWS, HALF1=HALF1, HALF2=HALF2,
        T1=T1.tolist(), T2=T2.tolist(), NT1=NT1, NT2=NT2, NG1=NG1, NG2=NG2,
        DIM_IN=DIM_IN, GC1=GC1, DG=DG, D2=D2, DH=DH, H1=H1, H2=H2,
    )

    # --- shared (replicated) tensors ---
    featT = np.zeros((DIM_IN, NT1ROWS), BF)
    featT[:, :N] = feat.T.astype(BF)
    w1 = np.asarray(inputs["gc1_W"], np.float32).astype(BF)          # [128,100]
    w2 = np.asarray(inputs["gc2_W"], np.float32).astype(BF)          # [100,20]
    iota = np.tile(np.arange(P, dtype=np.float32), (P, 1))
    ident = np.eye(P, dtype=np.float32)
    b1b = np.tile(np.asarray(inputs["gc1_b"], np.float32), (P, 1))   # [128,100]
    b2b = np.tile(np.asarray(inputs["gc2_b"], np.float32), (P, 1))   # [128,20]

    pg_W = np.asarray(inputs["pg_W"], np.float32); pg_b = np.asarray(inputs["pg_b"], np.float32)
    p2_W = np.asarray(inputs["p2_W"], np.float32); p2_b = np.asarray(inputs["p2_b"], np.float32)
    W2m = np.asarray(inputs["W2"], np.float32)
    w2eff = np.concatenate([pg_W, pg_b[None, :]], 0) @ W2m            # [21, 64]
    p2w_aug = np.concatenate([p2_W, p2_b[None, :]], 0)                # [201, 64]
    FD = (DG + 1) * (D2 + 1)
    FDP = -(-FD // P) * P
    fc1w = np.zeros((FDP, H1), np.float32)
    fc1w[:FD] = np.asarray(inputs["fc1_W"], np.float32)
    fc1b_r = np.asarray(inputs["fc1_b"], np.float32)
    fc2w = np.asarray(inputs["fc2_W"], np.float32)
    fc2b_r = np.asarray(inputs["fc2_b"], np.float32)
    fc3w = np.asarray(inputs["fc3_W"], np.float32)
    fc3b_r = np.asarray(inputs["fc3_b"], np.float32)
    bn1g = np.asarray(inputs["bn1_g"], np.float32)[:, None]
    bn1b = np.asarray(inputs["bn1_b"], np.float32)[:, None]
    bn2g = np.asarray(inputs["bn2_g"], np.float32)[:, None]
    bn2b = np.asarray(inputs["bn2_b"], np.float32)[:, None]
    # fc biases ride along as [H,1] columns added before BN.
    # BN(x+c) absorbs additive consts into the mean, but relu(out@fc3+b) does
    # not, so fc1_b/fc2_b only matter through BN: BN(x + b) == BN(x) exactly
    # (mean shifts by b). So fc1_b and fc2_b cancel entirely; fc3_b survives.
    meta["FDP"] = FDP
    desc2d = np.asarray(inputs["desc_2d"], np.float32)                # [B, 200]

    per_core = []
    for c in range(NC):
        buckets = per_core_edges[c]
        idx1 = np.zeros((max(NG1, 1), P, NI // 16), np.int16)
        dr1 = np.zeros((P, NT1), np.float32)
        idx2 = np.zeros((max(NG2, 1), P, NI // 16), np.int16)
        dr2 = np.zeros((P, NT2), np.float32)
        for (L, idx_arr, dr_arr, T) in (("L1", idx1, dr1, T1), ("L2", idx2, dr2, T2)):
            g_i = 0
            t_i = 0
            for b in range(B_blk):
                for h in (0, 1):
                    nt = int(T[b, h])
                    ii, dd = buckets[(L, b, h)]
                    iidx, ddr = _bucket_plan(ii, dd, h, ii, nt)
                    # per-tile dstrel columns (partition-major)
                    dr_arr[:, t_i:t_i + nt] = ddr.reshape(nt, P).T
                    t_i += nt
                    # gather groups of up to TG tiles (last group exact-sized)
                    for g0 in range(0, nt, TG):
                        gtiles = min(TG, nt - g0)
                        flat = iidx[g0 * P:(g0 + gtiles) * P]
                        idx_arr[g_i, :, : gtiles * P // 16] = _wrap_idx(flat)
                        g_i += 1
        nloc = int(nodes_c[c])
        rdeg = np.ones((B_blk * P,), np.float32)
        rdeg[:nloc] = rdeg_full[g_start[c]:g_end[c]]
        gidrel = np.full((B_blk * P,), 255.0, np.float32)
        gidrel[:nloc] = (gid[g_start[c]:g_end[c]] - c * GPC).astype(np.float32)
        rcnt = rcnt_full[c * GPC:(c + 1) * GPC][:, None]              # [GPC,1]
        d2c = desc2d[c * GPC:(c + 1) * GPC]                            # [GPC,200]
        d2T_aug = np.concatenate([d2c.T, np.ones((1, GPC), np.float32)], 0)  # [201,GPC]
        per_core.append({
            "featT": featT, "w1": w1, "w2": w2, "iota": iota, "ident": ident,
            "b1b": b1b, "b2b": b2b,
            "idx1": idx1, "dr1": dr1, "idx2": idx2, "dr2": dr2,
            "rdeg": rdeg.reshape(B_blk, P).T.copy(),      # [128, B_blk]
            "gidrel": gidrel.reshape(B_blk, P).T.copy(),  # [128, B_blk]
            "rcnt": rcnt, "d2gm": d2c, "d2T": d2T_aug,
            "w2eff": w2eff, "p2w": p2w_aug,
            "fc1w": fc1w, "fc2w": fc2w, "fc3w": fc3w,
            "fc3b": np.array([[float(fc3b_r[0])]], np.float32),
            "bn1g": bn1g, "bn1b": bn1b, "bn2g": bn2g, "bn2b": bn2b,
        })
    return meta, per_core


# ----------------------------------------------------------------------------
# Device program
# ----------------------------------------------------------------------------

def build(meta):
    NC = meta["NC"]; B = meta["B"]; GPC = meta["GPC"]; B_blk = meta["B_blk"]
    NPAD = meta["NPAD"]; TOTPAD = meta["TOTPAD"]
    NT1ROWS = meta["NT1ROWS"]; HALF1 = meta["HALF1"]; HALF2 = meta["HALF2"]
    T1 = meta["T1"]; T2 = meta["T2"]; NT1 = meta["NT1"]; NT2 = meta["NT2"]
    NG1 = meta["NG1"]; NG2 = meta["NG2"]
    DIM_IN = meta["DIM_IN"]; GC1 = meta["GC1"]; DG = meta["DG"]; D2 = meta["D2"]
    H1 = meta["H1"]; H2 = meta["H2"]; FDP = meta["FDP"]; PH = meta["PH"]
    EQ = mybir.AluOpType.is_equal
    MUL = mybir.AluOpType.mult
    ADD = mybir.AluOpType.add
    SUB = mybir.AluOpType.subtract
    AF = mybir.ActivationFunctionType

    nc = bacc.Bacc("TRN2", target_bir_lowering=False, debug=False, num_devices=NC)

    def din(name, shape, dt):
        return nc.dram_tensor(name, shape, dt, kind="ExternalInput").ap()

    featT_d = din("featT", [DIM_IN, NT1ROWS], BF16)
    w1_d = din("w1", [DIM_IN, GC1], BF16)
    w2_d = din("w2", [GC1, DG], BF16)
    iota_d = din("iota", [P, P], F32)
    ident_d = din("ident", [P, P], F32)
    b1b_d = din("b1b", [P, GC1], F32)
    b2b_d = din("b2b", [P, DG], F32)
    idx1_d = din("idx1", [max(NG1, 1), P, NI // 16], I16)
    dr1_d = din("dr1", [P, NT1], F32)
    idx2_d = din("idx2", [max(NG2, 1), P, NI // 16], I16)
    dr2_d = din("dr2", [P, NT2], F32)
    rdeg_d = din("rdeg", [P, B_blk], F32)
    gidrel_d = din("gidrel", [P, B_blk], F32)
    rcnt_d = din("rcnt", [GPC, 1], F32)
    d2gm_d = din("d2gm", [GPC, D2], F32)
    d2T_d = din("d2T", [D2 + 1, GPC], F32)
    w2eff_d = din("w2eff", [DG + 1, 64], F32)
    p2w_d = din("p2w", [D2 + 1, 64], F32)
    fc1w_d = din("fc1w", [FDP, H1], F32)
    fc2w_d = din("fc2w", [H1, H2], F32)
    fc3w_d = din("fc3w", [H2, 1], F32)
    fc3b_d = din("fc3b", [1, 1], F32)
    bn1g_d = din("bn1g", [H1, 1], F32)
    bn1b_d = din("bn1b", [H1, 1], F32)
    bn2g_d = din("bn2g", [H2, 1], F32)
    bn2b_d = din("bn2b", [H2, 1], F32)

    t1_d = nc.dram_tensor("t1tab", [NT1ROWS, P], BF16).ap()
    t2sh_d = nc.dram_tensor("t2shard", [NPAD, 32], BF16).ap()
    t2full_d = nc.dram_tensor("t2full", [TOTPAD, 32], BF16, addr_space="Shared").ap()
    t2pad_d = nc.dram_tensor("t2pad", [TOTPAD, P], BF16).ap()
    bn1i_d = nc.dram_tensor("bn1i", [H1, 2], F32).ap()
    bn1o_d = nc.dram_tensor("bn1o", [H1, 2], F32, addr_space="Shared").ap()
    bn2i_d = nc.dram_tensor("bn2i", [H2, 2], F32).ap()
    bn2o_d = nc.dram_tensor("bn2o", [H2, 2], F32, addr_space="Shared").ap()
    out_d = nc.dram_tensor("out", [1, GPC], F32, kind="ExternalOutput").ap()

    groups = [list(range(NC))]

    class _SkipRest(Exception):
        pass

    with tile.TileContext(nc) as tc:
        from contextlib import ExitStack
        with ExitStack() as ctx:
          try:
            cp = ctx.enter_context(tc.tile_pool(name="consts", bufs=1))
            fpool = ctx.enter_context(tc.tile_pool(name="feat", bufs=3))
            pp_t1 = ctx.enter_context(tc.tile_pool(name="p_t1", bufs=2, space="PSUM"))
            sb_t1 = ctx.enter_context(tc.tile_pool(name="sb_t1", bufs=4))
            ip = ctx.enter_context(tc.tile_pool(name="idx", bufs=6))
            payp = ctx.enter_context(tc.tile_pool(name="pay", bufs=5))
            selp = ctx.enter_context(tc.tile_pool(name="sel", bufs=8))
            drp = ctx.enter_context(tc.tile_pool(name="dr", bufs=3))
            pp_agg = ctx.enter_context(tc.tile_pool(name="p_agg", bufs=2, space="PSUM"))
            pp_tr = ctx.enter_context(tc.tile_pool(name="p_tr", bufs=1, space="PSUM"))
            pp_t2 = ctx.enter_context(tc.tile_pool(name="p_t2", bufs=1, space="PSUM"))
            hpool = ctx.enter_context(tc.tile_pool(name="hwork", bufs=3))
            pp_hg = ctx.enter_context(tc.tile_pool(name="p_hg", bufs=1, space="PSUM"))
            hd = ctx.enter_context(tc.tile_pool(name="head", bufs=1))

            nc.gpsimd.load_library(_mlp_lib)

            # ---- constants ----
            iota_t = cp.tile([P, P], F32); nc.sync.dma_start(iota_t[:], iota_d[:])
            zcol = cp.tile([P, 1], F32); nc.vector.memset(zcol[:], 0.0)
            ident_t = cp.tile([P, P], F32); nc.sync.dma_start(ident_t[:], ident_d[:])
            w1_t = cp.tile([DIM_IN, GC1], BF16); nc.sync.dma_start(w1_t[:], w1_d[:])
            w2_t = cp.tile([GC1, DG], BF16); nc.sync.dma_start(w2_t[:], w2_d[:])
            b1b_t = cp.tile([P, GC1], F32); nc.sync.dma_start(b1b_t[:], b1b_d[:])
            b2b_t = cp.tile([P, DG], F32); nc.sync.dma_start(b2b_t[:], b2b_d[:])
            rdeg_t = cp.tile([P, B_blk], F32); nc.sync.dma_start(rdeg_t[:], rdeg_d[:])
            gidr_t = cp.tile([P, B_blk], F32); nc.sync.dma_start(gidr_t[:], gidrel_d[:])

            # ================= Phase 1: t1 = feat @ W1 (replicated) ==========
            _sc1 = nc.enter_named_scope("ph1_t1", False)
            n_nt = NT1ROWS // P
            CHUNK = 16
            for c0 in range(0, n_nt, CHUNK):
                cw = min(CHUNK, n_nt - c0)
                ft = fpool.tile([P, CHUNK * P], BF16, tag="featT")
                nc.sync.dma_start(ft[:, :cw * P], featT_d[:, c0 * P:(c0 + cw) * P])
                for t in range(cw):
                    ps = pp_t1.tile([P, GC1], F32, tag="t1ps")
                    nc.tensor.matmul(ps[:], lhsT=ft[:, t * P:(t + 1) * P],
                                     rhs=w1_t[:], start=True, stop=True)
                    sb = sb_t1.tile([P, P], BF16, tag="t1sb")
                    nc.vector.tensor_copy(sb[:, :GC1], ps[:])
                    nc.vector.memset(sb[:, GC1:], 0.0)
                    nt = c0 + t
                    nc.sync.dma_start(t1_d[nt * P:(nt + 1) * P, :], sb[:])

            # ================= Phase 2/5 shared edge-layer builder ===========
            def edge_layer(layer, tab_d, half, T, ng_base_unused, idx_dram, dr_dram,
                           ncols, bias_t, out_block):
                """Per dst-block: gather + one-hot matmul segsum + epilogue."""
                g_i = 0
                t_i = 0
                for b in range(B_blk):
                    Tb = int(T[b][0]) + int(T[b][1])
                    dr_sb = drp.tile([P, max(Tb, 1)], F32, tag="drsb")
                    nc.sync.dma_start(dr_sb[:, :Tb], dr_dram[:, t_i:t_i + Tb])
                    ps = pp_agg.tile([P, ncols], F32, tag="agg")
                    k = 0
                    for h in (0, 1):
                        nt = int(T[b][h])
                        for g0 in range(0, nt, TG):
                            gt = min(TG, nt - g0)
                            ni = gt * P
                            ix = ip.tile([P, NI // 16], I16, tag="ix")
                            nc.sync.dma_start(ix[:, :ni // 16], idx_dram[g_i, :, :ni // 16])
                            pay = payp.tile([P, TG, P], BF16, tag="pay")
                            hi = min((h + 1) * half, tab_d.shape[0])
                            nc.gpsimd.dma_gather(
                                pay[:, :gt, :], tab_d[h * half:hi, :], ix[:, :ni // 16],
                                ni, ni, P)
                            for cc in range(gt):
                                sel = selp.tile([P, P], BF16, tag="sel")
                                nc.vector.tensor_scalar(
                                    out=sel[:], in0=iota_t[:],
                                    scalar1=dr_sb[:, k:k + 1], scalar2=None,
                                    op0=EQ)
                                nc.tensor.matmul(
                                    ps[:], lhsT=sel[:], rhs=pay[:, cc, :ncols],
                                    start=(k == 0), stop=(k == Tb - 1))
                                k += 1
                            g_i += 1
                    t_i += Tb
                    out_block(b, ps)

            # ---- Phase 2: layer 1 ----
            def l1_out(b, ps):
                h1 = hpool.tile([P, GC1], F32, tag="h1")
                nc.vector.tensor_scalar(out=h1[:], in0=ps[:],
                                        scalar1=rdeg_t[:, b:b + 1], scalar2=None,
                                        op0=MUL)
                nc.vector.tensor_tensor(out=h1[:], in0=h1[:], in1=b1b_t[:], op=ADD)
                nc.scalar.activation(out=h1[:], in_=h1[:], func=AF.Relu, bias=zcol[:, :1])
                tp = pp_tr.tile([GC1, P], F32, tag="trp")
                nc.tensor.transpose(tp[:], h1[:], ident_t[:])
                h1T = hpool.tile([GC1, P], BF16, tag="h1T")
                nc.vector.tensor_copy(h1T[:], tp[:])
                t2p = pp_t2.tile([P, DG], F32, tag="t2p")
                nc.tensor.matmul(t2p[:], lhsT=h1T[:], rhs=w2_t[:], start=True, stop=True)
                t2s = sb_t1.tile([P, P], BF16, tag="t2s")
                nc.vector.tensor_copy(t2s[:, :DG], t2p[:])
                nc.vector.memset(t2s[:, DG:], 0.0)
                nc.sync.dma_start(t2sh_d[b * P:(b + 1) * P, :], t2s[:, :32])

            nc.leave_named_scope("ph1_t1", _sc1[0], False)
            _sc2 = nc.enter_named_scope("ph2_L1", False)
            if PH >= 2:
                edge_layer("1", t1_d, HALF1, T1, 0, idx1_d, dr1_d, GC1, b1b_t, l1_out)
            nc.leave_named_scope("ph2_L1", _sc2[0], False)

            # ---- Phase 3: AllGather t2 ----
            _sc3 = nc.enter_named_scope("ph3_ag", False)
            if PH >= 3:
              nc.gpsimd.collective_compute(
                "AllGather", mybir.AluOpType.bypass, replica_groups=groups,
                ins=[t2sh_d[:].opt()], outs=[t2full_d[:].opt()])

            nc.leave_named_scope("ph3_ag", _sc3[0], False)
            # ---- Phase 4: restride tight [*,32] -> padded [*,128] ----
            _sc4 = nc.enter_named_scope("ph4_restride", False)
            for i in range(TOTPAD // P if PH >= 4 else 0):
                rs = sb_t1.tile([P, P], BF16, tag="rs")
                nc.sync.dma_start(rs[:, :32], t2full_d[i * P:(i + 1) * P, :])
                nc.vector.memset(rs[:, 32:], 0.0)
                nc.sync.dma_start(t2pad_d[i * P:(i + 1) * P, :], rs[:])

            nc.leave_named_scope("ph4_restride", _sc4[0], False)
            # ---- Phase 5: layer 2 + pooling ----
            do_l2 = PH >= 5
            do_head = PH >= 6
            hg_ps = pp_hg.tile([GPC, DG], F32, tag="hgps")

            def l2_out(b, ps):
                h2t = hpool.tile([P, DG], F32, tag="h2")
                nc.vector.tensor_scalar(out=h2t[:], in0=ps[:],
                                        scalar1=rdeg_t[:, b:b + 1], scalar2=None,
                                        op0=MUL)
                nc.vector.tensor_tensor(out=h2t[:], in0=h2t[:], in1=b2b_t[:], op=ADD)
                nc.scalar.activation(out=h2t[:], in_=h2t[:], func=AF.Relu, bias=zcol[:P, :1])
                selg = selp.tile([P, GPC], F32, tag="selg")
                nc.vector.tensor_scalar(out=selg[:], in0=iota_t[:, :GPC],
                                        scalar1=gidr_t[:, b:b + 1], scalar2=None,
                                        op0=EQ)
                nc.tensor.matmul(hg_ps[:], lhsT=selg[:], rhs=h2t[:],
                                 start=(b == 0), stop=(b == B_blk - 1),
                                 skip_group_check=True)

            _sc5 = nc.enter_named_scope("ph5_L2", False)
            if do_l2:
                edge_layer("2", t2pad_d, HALF2, T2, NG1, idx2_d, dr2_d, DG, b2b_t, l2_out)
            nc.leave_named_scope("ph5_L2", _sc5[0], False)

            # ================= Head ==========================================
            if not do_head:
                raise _SkipRest()
            _sc6 = nc.enter_named_scope("ph6_head", False)
            rcnt_t = hd.tile([GPC, 1], F32); nc.sync.dma_start(rcnt_t[:], rcnt_d[:])
            d2gm_t = hd.tile([GPC, D2], F32); nc.sync.dma_start(d2gm_t[:], d2gm_d[:])
            d2T_a = hd.tile([P, GPC], F32); nc.sync.dma_start(d2T_a[:], d2T_d[:P, :])
            d2T_b = hd.tile([D2 + 1 - P, GPC], F32); nc.sync.dma_start(d2T_b[:], d2T_d[P:, :])
            w2e_t = hd.tile([DG + 1, 64], F32); nc.sync.dma_start(w2e_t[:], w2eff_d[:])
            p2w_a = hd.tile([P, 64], F32); nc.sync.dma_start(p2w_a[:], p2w_d[:P, :])
            p2w_b = hd.tile([D2 + 1 - P, 64], F32); nc.sync.dma_start(p2w_b[:], p2w_d[P:, :])
            fc1w_t = hd.tile([P, FDP // P, H1], F32)
            nc.sync.dma_start(fc1w_t[:], fc1w_d[:].rearrange("(c p) h -> p c h", p=P))
            fc2w_t = hd.tile([H1, H2], F32); nc.sync.dma_start(fc2w_t[:], fc2w_d[:])
            fc3w_t = hd.tile([H2, 1], F32); nc.sync.dma_start(fc3w_t[:], fc3w_d[:])
            fc3b_t = hd.tile([1, 1], F32); nc.sync.dma_start(fc3b_t[:], fc3b_d[:])
            bn1g_t = hd.tile([H1, 1], F32); nc.sync.dma_start(bn1g_t[:], bn1g_d[:])
            bn1b_t = hd.tile([H1, 1], F32); nc.sync.dma_start(bn1b_t[:], bn1b_d[:])
            bn2g_t = hd.tile([H2, 1], F32); nc.sync.dma_start(bn2g_t[:], bn2g_d[:])
            bn2b_t = hd.tile([H2, 1], F32); nc.sync.dma_start(bn2b_t[:], bn2b_d[:])
            if PH == 60:
                raise _SkipRest()

            # hg1 = [hg | 1]
            hg1 = hd.tile([GPC, DG + 1], F32)
            nc.vector.tensor_scalar(out=hg1[:, :DG], in0=hg_ps[:], scalar1=rcnt_t[:, :1],
                                    scalar2=None, op0=MUL)
            nc.vector.memset(hg1[:, DG:DG + 1], 1.0)
            # hgT
            tp2 = pp_tr.tile([DG + 1, GPC], F32, tag="trp")
            nc.tensor.transpose(tp2[:], hg1[:], ident_t[:GPC, :GPC])
            hgT = hd.tile([DG + 1, GPC], F32)
            nc.vector.tensor_copy(hgT[:], tp2[:])
            # h_gm, h_d (graph-major [GPC, 64])
            hgm_ps = pp_t1.tile([GPC, 64], F32, tag="t1ps")
            nc.tensor.matmul(hgm_ps[:], lhsT=hgT[:], rhs=w2e_t[:], start=True, stop=True)
            hdm_ps = pp_t1.tile([GPC, 64], F32, tag="t1ps")
            nc.tensor.matmul(hdm_ps[:], lhsT=d2T_a[:], rhs=p2w_a[:],
                             start=True, stop=False)
            nc.tensor.matmul(hdm_ps[:], lhsT=d2T_b[:], rhs=p2w_b[:],
                             start=False, stop=True)
            hgm_sb = hd.tile([GPC, 64], F32)
            nc.vector.tensor_copy(hgm_sb[:], hgm_ps[:])
            junk = hd.tile([GPC, 64], F32)
            s_t = hd.tile([GPC, 1], F32)
            nc.vector.tensor_tensor(out=junk[:], in0=hgm_sb[:], in1=hdm_ps[:], op=MUL)
            nc.vector.reduce_sum(out=s_t[:], in_=junk[:], axis=mybir.AxisListType.X)
            a_t = hd.tile([GPC, 1], F32)
            nc.scalar.activation(out=a_t[:], in_=s_t[:], func=AF.Sigmoid, bias=zcol[:GPC, :1])
            if PH == 61:
                raise _SkipRest()
            # d1 = [a * desc2d | 1]
            d1 = hd.tile([GPC, D2 + 1], F32)
            nc.vector.tensor_scalar(out=d1[:, :D2], in0=d2gm_t[:], scalar1=a_t[:, :1],
                                    scalar2=None, op0=MUL)
            nc.vector.memset(d1[:, D2:D2 + 1], 1.0)
            # fusion [GPC, FDP]
            fus = hd.tile([GPC, FDP], F32)
            for i in range(DG + 1):
                nc.vector.tensor_scalar(out=fus[:, i * (D2 + 1):(i + 1) * (D2 + 1)],
                                        in0=d1[:], scalar1=hg1[:, i:i + 1],
                                        scalar2=None, op0=MUL)
            FD = (DG + 1) * (D2 + 1)
            if FDP > FD:
                nc.vector.memset(fus[:, FD:], 0.0)
            if PH == 62:
                raise _SkipRest()
            # fc1 (feature-major out [H1, GPC])
            fc1_ps = pp_t1.tile([H1, GPC], F32, tag="t1ps")
            for kt in range(FDP // P):
                ftp = pp_tr.tile([P, GPC], F32, tag="trp")
                nc.tensor.transpose(ftp[:], fus[:, kt * P:(kt + 1) * P],
                                    ident_t[:GPC, :GPC])
                fT = hpool.tile([P, GPC], F32, tag="fT")
                nc.vector.tensor_copy(fT[:], ftp[:])
                nc.tensor.matmul(fc1_ps[:], lhsT=fc1w_t[:, kt, :], rhs=fT[:],
                                 start=(kt == 0), stop=(kt == FDP // P - 1),
                                 skip_group_check=True)

            if PH < 7:
                raise _SkipRest()

            def bn_relu(x_ps, Hdim, g_t, b_t, bni_d, bno_d, tagp):
                xsb = hd.tile([Hdim, GPC], F32, name=f"xsb{tagp}")
                nc.vector.tensor_copy(xsb[:], x_ps[:])
                sums = hd.tile([Hdim, 1], F32, name=f"sums{tagp}")
                nc.vector.reduce_sum(out=sums[:], in_=xsb[:], axis=mybir.AxisListType.X)
                sqj = hd.tile([Hdim, GPC], F32, name=f"sqj{tagp}")
                sumsq = hd.tile([Hdim, 1], F32, name=f"sumsq{tagp}")
                nc.vector.tensor_tensor(out=sqj[:], in0=xsb[:], in1=xsb[:], op=MUL)
                nc.vector.reduce_sum(out=sumsq[:], in_=sqj[:], axis=mybir.AxisListType.X)
                stat = hd.tile([Hdim, 2], F32, name=f"stat{tagp}")
                nc.vector.tensor_copy(stat[:, 0:1], sums[:])
                nc.vector.tensor_copy(stat[:, 1:2], sumsq[:])
                nc.sync.dma_start(bni_d[:], stat[:])
                nc.gpsimd.collective_compute(
                    "AllReduce", ADD, replica_groups=groups,
                    ins=[bni_d[:].opt()], outs=[bno_d[:].opt()])
                statg = hd.tile([Hdim, 2], F32, name=f"statg{tagp}")
                nc.sync.dma_start(statg[:], bno_d[:])
                mean = hd.tile([Hdim, 1], F32, name=f"mean{tagp}")
                nc.vector.tensor_scalar(out=mean[:], in0=statg[:, 0:1],
                                        scalar1=1.0 / B, scalar2=None, op0=MUL)
                var = hd.tile([Hdim, 1], F32, name=f"var{tagp}")
                nc.vector.tensor_scalar(out=var[:], in0=statg[:, 1:2],
                                        scalar1=1.0 / B, scalar2=None, op0=MUL)
                msq = hd.tile([Hdim, 1], F32, name=f"msq{tagp}")
                nc.vector.tensor_tensor(out=msq[:], in0=mean[:], in1=mean[:], op=MUL)
                nc.vector.tensor_tensor(out=var[:], in0=var[:], in1=msq[:], op=SUB)
                nc.vector.tensor_scalar(out=var[:], in0=var[:], scalar1=BN_EPS,
                                        scalar2=None, op0=ADD)
                sd = hd.tile([Hdim, 1], F32, name=f"sd{tagp}")
                nc.scalar.activation(out=sd[:], in_=var[:], func=AF.Sqrt, bias=zcol[:Hdim, :1])
                rsd = hd.tile([Hdim, 1], F32, name=f"rsd{tagp}")
                nc.vector.reciprocal(rsd[:], sd[:])
                scl = hd.tile([Hdim, 1], F32, name=f"scl{tagp}")
                nc.vector.tensor_tensor(out=scl[:], in0=rsd[:], in1=g_t[:], op=MUL)
                tb = hd.tile([Hdim, 1], F32, name=f"tb{tagp}")
                nc.vector.tensor_tensor(out=tb[:], in0=mean[:], in1=scl[:], op=MUL)
                nc.vector.tensor_scalar(out=tb[:], in0=tb[:], scalar1=-1.0,
                                        scalar2=None, op0=MUL)
                nc.vector.tensor_tensor(out=tb[:], in0=tb[:], in1=b_t[:], op=ADD)
                o = hd.tile([Hdim, GPC], F32, name=f"bno{tagp}")
                nc.scalar.activation(out=o[:], in_=xsb[:], func=AF.Relu,
                                     bias=tb[:, 0:1], scale=scl[:, 0:1])
                return o

            bn1o_t = bn_relu(fc1_ps, H1, bn1g_t, bn1b_t, bn1i_d, bn1o_d, "1")
            if PH < 8:
                raise _SkipRest()
            fc2_ps = pp_t1.tile([H2, GPC], F32, tag="t1ps")
            nc.tensor.matmul(fc2_ps[:], lhsT=fc2w_t[:], rhs=bn1o_t[:], start=True, stop=True)
            bn2o_t = bn_relu(fc2_ps, H2, bn2g_t, bn2b_t, bn2i_d, bn2o_d, "2")
            fc3_ps = pp_t1.tile([1, GPC], F32, tag="t1ps")
            nc.tensor.matmul(fc3_ps[:], lhsT=fc3w_t[:], rhs=bn2o_t[:], start=True, stop=True)
            outsb = hd.tile([1, GPC], F32)
            nc.vector.tensor_scalar(out=outsb[:], in0=fc3_ps[:],
                                    scalar1=fc3b_t[0:1, 0:1], scalar2=None, op0=ADD)
            nc.sync.dma_start(out_d[:], outsb[:])
            nc.leave_named_scope("ph6_head", _sc6[0], False)
          except _SkipRest:
            pass

    nc.compile()
    return nc


# ----------------------------------------------------------------------------
# Entry point
# ----------------------------------------------------------------------------

REAL_DIMS = dict(N=50000, E=800000, B=512, DIM_IN=128, GC1=100, DG=20,
                 D2=200, DH=64, H1=128, H2=32)
_CACHE = {}


def run(inputs, nc_cores=8, dims=None, trace=False):
    dims = dims or REAL_DIMS
    meta, per_core = plan(inputs, nc_cores, dims)
    key = repr(sorted(meta.items()))
    if key not in _CACHE:
        _CACHE[key] = build(meta)
    prog = _CACHE[key]
    from concourse.bass_utils import run_bass_kernel_spmd
    res = run_bass_kernel_spmd(prog, per_core, list(range(nc_cores)), trace=trace)
    outs = [np.asarray(res.results[c]["out"]).reshape(-1) for c in range(nc_cores)]
    y = np.concatenate(outs).astype(np.float32)[:, None]
    return y, res


def kernel(**inputs):
    y, _ = run(inputs, nc_cores=8, dims=REAL_DIMS, trace=False)
    return y


# revision 18
# speedup vs baseline: 1.1029x; 1.0793x over previous
"""Trainium2 Bass kernel for nn_BiAttn_TFN_hg_2desc_Net (GNN message passing).

Strategy (8 NeuronCores, SPMD single program):
  - Nodes/graphs sharded by graph (64 graphs/core, contiguous node ranges since
    graph_id is sorted). Edges sharded by dst-owner core.
  - Phase 1 (replicated): t1 = feat @ gc1_W, computed fully on every core from
    a host-pretransposed featT (bf16). Replication avoids any collective here.
  - Phase 2 (L1 edge aggregation): edges bucketed host-side by
    (dst 128-block, table half); payload rows t1[src] fetched with dma_gather
    (int16 idx, 1024 rows/instruction); segment-sum by dst done with one-hot
    selector matmuls (selector built on-device: is_equal(iota, dstrel)) into
    PSUM; epilogue h1 = relu(agg*rdeg + b1) with host-precomputed 1/deg, then
    t2 = h1 @ gc2_W via a PE transpose.
  - Phase 3: one AllGather of the tight t2 (bf16) across the 8 cores.
  - Phase 4: restride t2 to 256B rows for gathering.
  - Phase 5 (L2 edge aggregation): same machinery, 20-wide payload, then
    graph-mean pooling via one-hot graph-selector matmuls (counts from host).
  - Head: bilinear attention + fusion outer-product + 3-layer MLP with
    BatchNorm, computed feature-major; BN batch stats via two tiny AllReduces.
"""

import sys

sys.path.insert(0, "/opt/trn_rl_repo")

import numpy as np
import ml_dtypes

import concourse.bass as bass
import concourse.bacc as bacc
import concourse.tile as tile
from concourse import mybir
from concourse import bass_utils
from concourse.library_config import mlp as _mlp_lib

bass_utils.upload_artifacts = lambda tmpdir: "local://skipped"

P = 128
TG = 8          # tiles per dma_gather (1024 indices)
NI = P * TG
BN_EPS = 1e-5

F32 = mybir.dt.float32
BF16 = mybir.dt.bfloat16
I16 = mybir.dt.int16

BF = ml_dtypes.bfloat16


# ----------------------------------------------------------------------------
# Host-side planning
# ----------------------------------------------------------------------------

def _wrap_idx(flat_idx):
    """[NI] int -> [128, NI//16] int16 in the dma_gather wrapped layout
    (idx i at [i % 16, i // 16], tiled x8 down the partitions)."""
    a = np.asarray(flat_idx, np.int16).reshape(-1, 16).T      # [16, NI/16]
    return np.tile(a, (8, 1))                                  # [128, NI/16]


def _bucket_plan(src_list, dstrel_list, half_of, idx_in_half, n_tiles):
    """Pad one (block, half) bucket's edges to n_tiles*128 and emit per-tile
    idx (int32 within half) and dstrel (f32) arrays."""
    e = len(src_list)
    tot = n_tiles * P
    idx = np.zeros(tot, np.int64)
    dr = np.full(tot, 255.0, np.float32)
    if e:
        idx[:e] = idx_in_half
        dr[:e] = dstrel_list
    return idx, dr


def plan(inputs, nc_cores, dims):
    """Host preprocessing. Returns (meta, per_core_inputs)."""
    N = dims["N"]; E = dims["E"]; B = dims["B"]
    DIM_IN = dims["DIM_IN"]; GC1 = dims["GC1"]; DG = dims["DG"]
    D2 = dims["D2"]; DH = dims["DH"]; H1 = dims["H1"]; H2 = dims["H2"]
    NC = nc_cores
    GPC = B // NC

    feat = np.asarray(inputs["feat"], np.float32)
    src = np.asarray(inputs["src"], np.int64)
    dst = np.asarray(inputs["dst"], np.int64)
    gid = np.asarray(inputs["graph_id"], np.int64)

    # --- core node/graph ranges (graph-aligned) ---
    bounds = np.searchsorted(gid, np.arange(0, B + 1, GPC))
    g_start, g_end = bounds[:-1], bounds[1:]
    nodes_c = g_end - g_start
    B_blk = int(np.ceil(nodes_c.max() / P))
    NPAD = B_blk * P
    TOTPAD = NC * NPAD
    # t1 table halves (rows of t1, global node ids, padded to 128)
    NT1ROWS = int(np.ceil(N / P)) * P
    HALF1 = (NT1ROWS // P // 2 + (NT1ROWS // P) % 2) * P
    assert HALF1 < 32768 and NT1ROWS - HALF1 < 32768
    HALF2 = TOTPAD // 2
    assert HALF2 % P == 0 and HALF2 < 32768

    # --- degrees / counts ---
    deg = np.bincount(dst, minlength=N).astype(np.float32)
    rdeg_full = 1.0 / np.maximum(deg, 1.0)
    cnt = np.bincount(gid, minlength=B).astype(np.float32)
    rcnt_full = (1.0 / np.maximum(cnt, 1.0)).astype(np.float32)

    # --- edge assignment ---
    core_of_dst = np.searchsorted(g_end - 1, dst)          # g_start <= dst < g_end
    # L2 gather index: position of src in the padded allgathered table
    core_of_src = np.searchsorted(g_end - 1, src)
    src_pad = core_of_src * NPAD + (src - g_start[core_of_src])

    per_core_edges = []
    T1 = np.zeros((B_blk, 2), np.int64)
    T2 = np.zeros((B_blk, 2), np.int64)
    for c in range(NC):
        m = core_of_dst == c
        e_src, e_dst, e_srcpad = src[m], dst[m], src_pad[m]
        drel = e_dst - g_start[c]
        blk = drel // P
        drel_in = (drel % P).astype(np.float32)
        h1b = (e_src >= HALF1).astype(np.int64)
        h2b = (e_srcpad >= HALF2).astype(np.int64)
        buckets = {}
        for b in range(B_blk):
            mb = blk == b
            for h in (0, 1):
                m1 = mb & (h1b == h)
                buckets[("L1", b, h)] = (e_src[m1] - h * HALF1, drel_in[m1])
                T1[b, h] = max(T1[b, h], int(np.ceil(m1.sum() / P)))
                m2 = mb & (h2b == h)
                buckets[("L2", b, h)] = (e_srcpad[m2] - h * HALF2, drel_in[m2])
                T2[b, h] = max(T2[b, h], int(np.ceil(m2.sum() / P)))
        per_core_edges.append(buckets)
    T1 = np.maximum(T1, 1)   # keep >= 1 tile so psum groups are never empty
    T2 = np.maximum(T2, 1)

    NT1 = int(T1.sum()); NT2 = int(T2.sum())
    NG1 = int(sum(-(-int(T1[b, h]) // TG) for b in range(B_blk) for h in (0, 1)))
    NG2 = int(sum(-(-int(T2[b, h]) // TG) for b in range(B_blk) for h in (0, 1)))

    import os
    meta = dict(
        PH=int(os.environ.get("K_PHASES", "9")),
        NC=NC, B=B, GPC=GPC, B_blk=B_blk, NPAD=NPAD, TOTPAD=TOTPAD,
        NT1ROWS=NT1ROWS, HALF1=HALF1, HALF2=HALF2,
        T1=T1.tolist(), T2=T2.tolist(), NT1=NT1, NT2=NT2, NG1=NG1, NG2=NG2,
        DIM_IN=DIM_IN, GC1=GC1, DG=DG, D2=D2, DH=DH, H1=H1, H2=H2,
    )

    # --- shared (replicated) tensors ---
    featT = np.zeros((DIM_IN, NT1ROWS), BF)
    featT[:, :N] = feat.T.astype(BF)
    w1 = np.asarray(inputs["gc1_W"], np.float32).astype(BF)          # [128,100]
    w2 = np.asarray(inputs["gc2_W"], np.float32).astype(BF)          # [100,20]
    iota = np.tile(np.arange(P, dtype=np.float32), (P, 1))
    ident = np.eye(P, dtype=np.float32)
    b1b = np.tile(np.asarray(inputs["gc1_b"], np.float32), (P, 1))   # [128,100]
    b2b = np.tile(np.asarray(inputs["gc2_b"], np.float32), (P, 1))   # [128,20]

    pg_W = np.asarray(inputs["pg_W"], np.float32); pg_b = np.asarray(inputs["pg_b"], np.float32)
    p2_W = np.asarray(inputs["p2_W"], np.float32); p2_b = np.asarray(inputs["p2_b"], np.float32)
    W2m = np.asarray(inputs["W2"], np.float32)
    w2eff = np.concatenate([pg_W, pg_b[None, :]], 0) @ W2m            # [21, 64]
    p2w_aug = np.concatenate([p2_W, p2_b[None, :]], 0)                # [201, 64]
    FD = (DG + 1) * (D2 + 1)
    FDP = -(-FD // P) * P
    fc1w = np.zeros((FDP, H1), np.float32)
    fc1w[:FD] = np.asarray(inputs["fc1_W"], np.float32)
    fc1b_r = np.asarray(inputs["fc1_b"], np.float32)
    fc2w = np.asarray(inputs["fc2_W"], np.float32)
    fc2b_r = np.asarray(inputs["fc2_b"], np.float32)
    fc3w = np.asarray(inputs["fc3_W"], np.float32)
    fc3b_r = np.asarray(inputs["fc3_b"], np.float32)
    bn1g = np.asarray(inputs["bn1_g"], np.float32)[:, None]
    bn1b = np.asarray(inputs["bn1_b"], np.float32)[:, None]
    bn2g = np.asarray(inputs["bn2_g"], np.float32)[:, None]
    bn2b = np.asarray(inputs["bn2_b"], np.float32)[:, None]
    # fc biases ride along as [H,1] columns added before BN.
    # BN(x+c) absorbs additive consts into the mean, but relu(out@fc3+b) does
    # not, so fc1_b/fc2_b only matter through BN: BN(x + b) == BN(x) exactly
    # (mean shifts by b). So fc1_b and fc2_b cancel entirely; fc3_b survives.
    meta["FDP"] = FDP
    desc2d = np.asarray(inputs["desc_2d"], np.float32)                # [B, 200]

    per_core = []
    for c in range(NC):
        buckets = per_core_edges[c]
        idx1 = np.zeros((max(NG1, 1), P, NI // 16), np.int16)
        dr1 = np.zeros((P, NT1), np.float32)
        idx2 = np.zeros((max(NG2, 1), P, NI // 16), np.int16)
        dr2 = np.zeros((P, NT2), np.float32)
        for (L, idx_arr, dr_arr, T) in (("L1", idx1, dr1, T1), ("L2", idx2, dr2, T2)):
            g_i = 0
            t_i = 0
            for b in range(B_blk):
                for h in (0, 1):
                    nt = int(T[b, h])
                    ii, dd = buckets[(L, b, h)]
                    iidx, ddr = _bucket_plan(ii, dd, h, ii, nt)
                    # per-tile dstrel columns (partition-major)
                    dr_arr[:, t_i:t_i + nt] = ddr.reshape(nt, P).T
                    t_i += nt
                    # gather groups of up to TG tiles (last group exact-sized)
                    for g0 in range(0, nt, TG):
                        gtiles = min(TG, nt - g0)
                        flat = iidx[g0 * P:(g0 + gtiles) * P]
                        idx_arr[g_i, :, : gtiles * P // 16] = _wrap_idx(flat)
                        g_i += 1
        nloc = int(nodes_c[c])
        rdeg = np.ones((B_blk * P,), np.float32)
        rdeg[:nloc] = rdeg_full[g_start[c]:g_end[c]]
        gidrel = np.full((B_blk * P,), 255.0, np.float32)
        gidrel[:nloc] = (gid[g_start[c]:g_end[c]] - c * GPC).astype(np.float32)
        rcnt = rcnt_full[c * GPC:(c + 1) * GPC][:, None]              # [GPC,1]
        d2c = desc2d[c * GPC:(c + 1) * GPC]                            # [GPC,200]
        d2T_aug = np.concatenate([d2c.T, np.ones((1, GPC), np.float32)], 0)  # [201,GPC]
        per_core.append({
            "featT": featT, "w1": w1, "w2": w2, "iota": iota, "ident": ident,
            "b1b": b1b, "b2b": b2b,
            "idx1": idx1, "dr1": dr1, "idx2": idx2, "dr2": dr2,
            "rdeg": rdeg.reshape(B_blk, P).T.copy(),      # [128, B_blk]
            "gidrel": gidrel.reshape(B_blk, P).T.copy(),  # [128, B_blk]
            "rcnt": rcnt, "d2gm": d2c, "d2T": d2T_aug,
            "w2eff": w2eff, "p2w": p2w_aug,
            "fc1w": fc1w, "fc2w": fc2w, "fc3w": fc3w,
            "fc3b": np.array([[float(fc3b_r[0])]], np.float32),
            "bn1g": bn1g, "bn1b": bn1b, "bn2g": bn2g, "bn2b": bn2b,
        })
    return meta, per_core


# ----------------------------------------------------------------------------
# Device program
# ----------------------------------------------------------------------------

def build(meta):
    NC = meta["NC"]; B = meta["B"]; GPC = meta["GPC"]; B_blk = meta["B_blk"]
    NPAD = meta["NPAD"]; TOTPAD = meta["TOTPAD"]
    NT1ROWS = meta["NT1ROWS"]; HALF1 = meta["HALF1"]; HALF2 = meta["HALF2"]
    T1 = meta["T1"]; T2 = meta["T2"]; NT1 = meta["NT1"]; NT2 = meta["NT2"]
    NG1 = meta["NG1"]; NG2 = meta["NG2"]
    DIM_IN = meta["DIM_IN"]; GC1 = meta["GC1"]; DG = meta["DG"]; D2 = meta["D2"]
    H1 = meta["H1"]; H2 = meta["H2"]; FDP = meta["FDP"]; PH = meta["PH"]
    EQ = mybir.AluOpType.is_equal
    MUL = mybir.AluOpType.mult
    ADD = mybir.AluOpType.add
    SUB = mybir.AluOpType.subtract
    AF = mybir.ActivationFunctionType

    nc = bacc.Bacc("TRN2", target_bir_lowering=False, debug=False, num_devices=NC)

    def din(name, shape, dt):
        return nc.dram_tensor(name, shape, dt, kind="ExternalInput").ap()

    featT_d = din("featT", [DIM_IN, NT1ROWS], BF16)
    w1_d = din("w1", [DIM_IN, GC1], BF16)
    w2_d = din("w2", [GC1, DG], BF16)
    iota_d = din("iota", [P, P], F32)
    ident_d = din("ident", [P, P], F32)
    b1b_d = din("b1b", [P, GC1], F32)
    b2b_d = din("b2b", [P, DG], F32)
    idx1_d = din("idx1", [max(NG1, 1), P, NI // 16], I16)
    dr1_d = din("dr1", [P, NT1], F32)
    idx2_d = din("idx2", [max(NG2, 1), P, NI // 16], I16)
    dr2_d = din("dr2", [P, NT2], F32)
    rdeg_d = din("rdeg", [P, B_blk], F32)
    gidrel_d = din("gidrel", [P, B_blk], F32)
    rcnt_d = din("rcnt", [GPC, 1], F32)
    d2gm_d = din("d2gm", [GPC, D2], F32)
    d2T_d = din("d2T", [D2 + 1, GPC], F32)
    w2eff_d = din("w2eff", [DG + 1, 64], F32)
    p2w_d = din("p2w", [D2 + 1, 64], F32)
    fc1w_d = din("fc1w", [FDP, H1], F32)
    fc2w_d = din("fc2w", [H1, H2], F32)
    fc3w_d = din("fc3w", [H2, 1], F32)
    fc3b_d = din("fc3b", [1, 1], F32)
    bn1g_d = din("bn1g", [H1, 1], F32)
    bn1b_d = din("bn1b", [H1, 1], F32)
    bn2g_d = din("bn2g", [H2, 1], F32)
    bn2b_d = din("bn2b", [H2, 1], F32)

    t1_d = nc.dram_tensor("t1tab", [NT1ROWS, P], BF16).ap()
    t2sh_d = nc.dram_tensor("t2shard", [NPAD, 32], BF16).ap()
    t2full_d = nc.dram_tensor("t2full", [TOTPAD, 32], BF16, addr_space="Shared").ap()
    t2pad_d = nc.dram_tensor("t2pad", [TOTPAD, P], BF16).ap()
    bn1i_d = nc.dram_tensor("bn1i", [H1, 2], F32).ap()
    bn1o_d = nc.dram_tensor("bn1o", [H1, 2], F32, addr_space="Shared").ap()
    bn2i_d = nc.dram_tensor("bn2i", [H2, 2], F32).ap()
    bn2o_d = nc.dram_tensor("bn2o", [H2, 2], F32, addr_space="Shared").ap()
    out_d = nc.dram_tensor("out", [1, GPC], F32, kind="ExternalOutput").ap()

    groups = [list(range(NC))]

    class _SkipRest(Exception):
        pass

    with tile.TileContext(nc) as tc:
        from contextlib import ExitStack
        with ExitStack() as ctx:
          try:
            cp = ctx.enter_context(tc.tile_pool(name="consts", bufs=1))
            fpool = ctx.enter_context(tc.tile_pool(name="feat", bufs=3))
            pp_t1 = ctx.enter_context(tc.tile_pool(name="p_t1", bufs=2, space="PSUM"))
            sb_t1 = ctx.enter_context(tc.tile_pool(name="sb_t1", bufs=4))
            ip = ctx.enter_context(tc.tile_pool(name="idx", bufs=6))
            payp = ctx.enter_context(tc.tile_pool(name="pay", bufs=5))
            selp = ctx.enter_context(tc.tile_pool(name="sel", bufs=8))
            drp = ctx.enter_context(tc.tile_pool(name="dr", bufs=3))
            pp_agg = ctx.enter_context(tc.tile_pool(name="p_agg", bufs=2, space="PSUM"))
            pp_tr = ctx.enter_context(tc.tile_pool(name="p_tr", bufs=1, space="PSUM"))
            pp_t2 = ctx.enter_context(tc.tile_pool(name="p_t2", bufs=1, space="PSUM"))
            hpool = ctx.enter_context(tc.tile_pool(name="hwork", bufs=3))
            pp_hg = ctx.enter_context(tc.tile_pool(name="p_hg", bufs=1, space="PSUM"))
            hd = ctx.enter_context(tc.tile_pool(name="head", bufs=1))

            nc.gpsimd.load_library(_mlp_lib)

            # ---- constants ----
            iota_t = cp.tile([P, P], F32); nc.sync.dma_start(iota_t[:], iota_d[:])
            zcol = cp.tile([P, 1], F32); nc.vector.memset(zcol[:], 0.0)
            ident_t = cp.tile([P, P], F32); nc.sync.dma_start(ident_t[:], ident_d[:])
            w1_t = cp.tile([DIM_IN, GC1], BF16); nc.sync.dma_start(w1_t[:], w1_d[:])
            w2_t = cp.tile([GC1, DG], BF16); nc.sync.dma_start(w2_t[:], w2_d[:])
            b1b_t = cp.tile([P, GC1], F32); nc.sync.dma_start(b1b_t[:], b1b_d[:])
            b2b_t = cp.tile([P, DG], F32); nc.sync.dma_start(b2b_t[:], b2b_d[:])
            rdeg_t = cp.tile([P, B_blk], F32); nc.sync.dma_start(rdeg_t[:], rdeg_d[:])
            gidr_t = cp.tile([P, B_blk], F32); nc.sync.dma_start(gidr_t[:], gidrel_d[:])

            # ================= Phase 1: t1 = feat @ W1 (replicated) ==========
            _sc1 = nc.enter_named_scope("ph1_t1", False)
            n_nt = NT1ROWS // P
            CHUNK = 16
            for c0 in range(0, n_nt, CHUNK):
                cw = min(CHUNK, n_nt - c0)
                ft = fpool.tile([P, CHUNK * P], BF16, tag="featT")
                nc.sync.dma_start(ft[:, :cw * P], featT_d[:, c0 * P:(c0 + cw) * P])
                stage = sb_t1.tile([P, CHUNK, P], BF16, tag="t1stage")
                for t in range(cw):
                    ps = pp_t1.tile([P, GC1], F32, tag="t1ps")
                    nc.tensor.matmul(ps[:], lhsT=ft[:, t * P:(t + 1) * P],
                                     rhs=w1_t[:], start=True, stop=True)
                    nc.vector.tensor_copy(stage[:, t, :GC1], ps[:])
                    nc.vector.memset(stage[:, t, GC1:], 0.0)
                nc.sync.dma_start(
                    t1_d[c0 * P:(c0 + cw) * P, :].rearrange(
                        "(c p) e -> p c e", c=cw, p=P),
                    stage[:, :cw, :])

            # ================= Phase 2/5 shared edge-layer builder ===========
            def edge_layer(layer, tab_d, half, T, ng_base_unused, idx_dram, dr_dram,
                           ncols, bias_t, out_block):
                """Per dst-block: gather + one-hot matmul segsum + epilogue."""
                g_i = 0
                t_i = 0
                for b in range(B_blk):
                    Tb = int(T[b][0]) + int(T[b][1])
                    dr_sb = drp.tile([P, max(Tb, 1)], F32, tag="drsb")
                    nc.sync.dma_start(dr_sb[:, :Tb], dr_dram[:, t_i:t_i + Tb])
                    ps = pp_agg.tile([P, ncols], F32, tag="agg")
                    k = 0
                    for h in (0, 1):
                        nt = int(T[b][h])
                        for g0 in range(0, nt, TG):
                            gt = min(TG, nt - g0)
                            ni = gt * P
                            ix = ip.tile([P, NI // 16], I16, tag="ix")
                            nc.sync.dma_start(ix[:, :ni // 16], idx_dram[g_i, :, :ni // 16])
                            pay = payp.tile([P, TG, P], BF16, tag="pay")
                            hi = min((h + 1) * half, tab_d.shape[0])
                            nc.gpsimd.dma_gather(
                                pay[:, :gt, :], tab_d[h * half:hi, :], ix[:, :ni // 16],
                                ni, ni, P)
                            for cc in range(gt):
                                sel = selp.tile([P, P], BF16, tag="sel")
                                nc.vector.tensor_scalar(
                                    out=sel[:], in0=iota_t[:],
                                    scalar1=dr_sb[:, k:k + 1], scalar2=None,
                                    op0=EQ)
                                nc.tensor.matmul(
                                    ps[:], lhsT=sel[:], rhs=pay[:, cc, :ncols],
                                    start=(k == 0), stop=(k == Tb - 1))
                                k += 1
                            g_i += 1
                    t_i += Tb
                    out_block(b, ps)

            # ---- Phase 2: layer 1 ----
            def l1_out(b, ps):
                h1 = hpool.tile([P, GC1], F32, tag="h1")
                nc.vector.tensor_scalar(out=h1[:], in0=ps[:],
                                        scalar1=rdeg_t[:, b:b + 1], scalar2=None,
                                        op0=MUL)
                nc.vector.tensor_tensor(out=h1[:], in0=h1[:], in1=b1b_t[:], op=ADD)
                nc.scalar.activation(out=h1[:], in_=h1[:], func=AF.Relu, bias=zcol[:, :1])
                tp = pp_tr.tile([GC1, P], F32, tag="trp")
                nc.tensor.transpose(tp[:], h1[:], ident_t[:])
                h1T = hpool.tile([GC1, P], BF16, tag="h1T")
                nc.vector.tensor_copy(h1T[:], tp[:])
                t2p = pp_t2.tile([P, DG], F32, tag="t2p")
                nc.tensor.matmul(t2p[:], lhsT=h1T[:], rhs=w2_t[:], start=True, stop=True)
                t2s = sb_t1.tile([P, P], BF16, tag="t2s")
                nc.vector.tensor_copy(t2s[:, :DG], t2p[:])
                nc.vector.memset(t2s[:, DG:], 0.0)
                nc.sync.dma_start(t2sh_d[b * P:(b + 1) * P, :], t2s[:, :32])

            nc.leave_named_scope("ph1_t1", _sc1[0], False)
            _sc2 = nc.enter_named_scope("ph2_L1", False)
            if PH >= 2:
                edge_layer("1", t1_d, HALF1, T1, 0, idx1_d, dr1_d, GC1, b1b_t, l1_out)
            nc.leave_named_scope("ph2_L1", _sc2[0], False)

            # ---- Phase 3: AllGather t2 ----
            _sc3 = nc.enter_named_scope("ph3_ag", False)
            if PH >= 3:
              nc.gpsimd.collective_compute(
                "AllGather", mybir.AluOpType.bypass, replica_groups=groups,
                ins=[t2sh_d[:].opt()], outs=[t2full_d[:].opt()])

            nc.leave_named_scope("ph3_ag", _sc3[0], False)
            # ---- Phase 4: restride tight [*,32] -> padded [*,128] ----
            _sc4 = nc.enter_named_scope("ph4_restride", False)
            for i in range(TOTPAD // P if PH >= 4 else 0):
                rs = sb_t1.tile([P, P], BF16, tag="rs")
                nc.sync.dma_start(rs[:, :32], t2full_d[i * P:(i + 1) * P, :])
                nc.vector.memset(rs[:, 32:], 0.0)
                nc.sync.dma_start(t2pad_d[i * P:(i + 1) * P, :], rs[:])

            nc.leave_named_scope("ph4_restride", _sc4[0], False)
            # ---- Phase 5: layer 2 + pooling ----
            do_l2 = PH >= 5
            do_head = PH >= 6
            hg_ps = pp_hg.tile([GPC, DG], F32, tag="hgps")

            def l2_out(b, ps):
                h2t = hpool.tile([P, DG], F32, tag="h2")
                nc.vector.tensor_scalar(out=h2t[:], in0=ps[:],
                                        scalar1=rdeg_t[:, b:b + 1], scalar2=None,
                                        op0=MUL)
                nc.vector.tensor_tensor(out=h2t[:], in0=h2t[:], in1=b2b_t[:], op=ADD)
                nc.scalar.activation(out=h2t[:], in_=h2t[:], func=AF.Relu, bias=zcol[:P, :1])
                selg = selp.tile([P, GPC], F32, tag="selg")
                nc.vector.tensor_scalar(out=selg[:], in0=iota_t[:, :GPC],
                                        scalar1=gidr_t[:, b:b + 1], scalar2=None,
                                        op0=EQ)
                nc.tensor.matmul(hg_ps[:], lhsT=selg[:], rhs=h2t[:],
                                 start=(b == 0), stop=(b == B_blk - 1),
                                 skip_group_check=True)

            _sc5 = nc.enter_named_scope("ph5_L2", False)
            if do_l2:
                edge_layer("2", t2pad_d, HALF2, T2, NG1, idx2_d, dr2_d, DG, b2b_t, l2_out)
            nc.leave_named_scope("ph5_L2", _sc5[0], False)

            # ================= Head ==========================================
            if not do_head:
                raise _SkipRest()
            _sc6 = nc.enter_named_scope("ph6_head", False)
            rcnt_t = hd.tile([GPC, 1], F32); nc.sync.dma_start(rcnt_t[:], rcnt_d[:])
            d2gm_t = hd.tile([GPC, D2], F32); nc.sync.dma_start(d2gm_t[:], d2gm_d[:])
            d2T_a = hd.tile([P, GPC], F32); nc.sync.dma_start(d2T_a[:], d2T_d[:P, :])
            d2T_b = hd.tile([D2 + 1 - P, GPC], F32); nc.sync.dma_start(d2T_b[:], d2T_d[P:, :])
            w2e_t = hd.tile([DG + 1, 64], F32); nc.sync.dma_start(w2e_t[:], w2eff_d[:])
            p2w_a = hd.tile([P, 64], F32); nc.sync.dma_start(p2w_a[:], p2w_d[:P, :])
            p2w_b = hd.tile([D2 + 1 - P, 64], F32); nc.sync.dma_start(p2w_b[:], p2w_d[P:, :])
            fc1w_t = hd.tile([P, FDP // P, H1], F32)
            nc.sync.dma_start(fc1w_t[:], fc1w_d[:].rearrange("(c p) h -> p c h", p=P))
            fc2w_t = hd.tile([H1, H2], F32); nc.sync.dma_start(fc2w_t[:], fc2w_d[:])
            fc3w_t = hd.tile([H2, 1], F32); nc.sync.dma_start(fc3w_t[:], fc3w_d[:])
            fc3b_t = hd.tile([1, 1], F32); nc.sync.dma_start(fc3b_t[:], fc3b_d[:])
            bn1g_t = hd.tile([H1, 1], F32); nc.sync.dma_start(bn1g_t[:], bn1g_d[:])
            bn1b_t = hd.tile([H1, 1], F32); nc.sync.dma_start(bn1b_t[:], bn1b_d[:])
            bn2g_t = hd.tile([H2, 1], F32); nc.sync.dma_start(bn2g_t[:], bn2g_d[:])
            bn2b_t = hd.tile([H2, 1], F32); nc.sync.dma_start(bn2b_t[:], bn2b_d[:])
            if PH == 60:
                raise _SkipRest()

            # hg1 = [hg | 1]
            hg1 = hd.tile([GPC, DG + 1], F32)
            nc.vector.tensor_scalar(out=hg1[:, :DG], in0=hg_ps[:], scalar1=rcnt_t[:, :1],
                                    scalar2=None, op0=MUL)
            nc.vector.memset(hg1[:, DG:DG + 1], 1.0)
            # hgT
            tp2 = pp_tr.tile([DG + 1, GPC], F32, tag="trp")
            nc.tensor.transpose(tp2[:], hg1[:], ident_t[:GPC, :GPC])
            hgT = hd.tile([DG + 1, GPC], F32)
            nc.vector.tensor_copy(hgT[:], tp2[:])
            # h_gm, h_d (graph-major [GPC, 64])
            hgm_ps = pp_t1.tile([GPC, 64], F32, tag="t1ps")
            nc.tensor.matmul(hgm_ps[:], lhsT=hgT[:], rhs=w2e_t[:], start=True, stop=True)
            hdm_ps = pp_t1.tile([GPC, 64], F32, tag="t1ps")
            nc.tensor.matmul(hdm_ps[:], lhsT=d2T_a[:], rhs=p2w_a[:],
                             start=True, stop=False)
            nc.tensor.matmul(hdm_ps[:], lhsT=d2T_b[:], rhs=p2w_b[:],
                             start=False, stop=True)
            hgm_sb = hd.tile([GPC, 64], F32)
            nc.vector.tensor_copy(hgm_sb[:], hgm_ps[:])
            junk = hd.tile([GPC, 64], F32)
            s_t = hd.tile([GPC, 1], F32)
            nc.vector.tensor_tensor(out=junk[:], in0=hgm_sb[:], in1=hdm_ps[:], op=MUL)
            nc.vector.reduce_sum(out=s_t[:], in_=junk[:], axis=mybir.AxisListType.X)
            a_t = hd.tile([GPC, 1], F32)
            nc.scalar.activation(out=a_t[:], in_=s_t[:], func=AF.Sigmoid, bias=zcol[:GPC, :1])
            if PH == 61:
                raise _SkipRest()
            # d1 = [a * desc2d | 1]
            d1 = hd.tile([GPC, D2 + 1], F32)
            nc.vector.tensor_scalar(out=d1[:, :D2], in0=d2gm_t[:], scalar1=a_t[:, :1],
                                    scalar2=None, op0=MUL)
            nc.vector.memset(d1[:, D2:D2 + 1], 1.0)
            # fusion [GPC, FDP]
            fus = hd.tile([GPC, FDP], F32)
            for i in range(DG + 1):
                nc.vector.tensor_scalar(out=fus[:, i * (D2 + 1):(i + 1) * (D2 + 1)],
                                        in0=d1[:], scalar1=hg1[:, i:i + 1],
                                        scalar2=None, op0=MUL)
            FD = (DG + 1) * (D2 + 1)
            if FDP > FD:
                nc.vector.memset(fus[:, FD:], 0.0)
            if PH == 62:
                raise _SkipRest()
            # fc1 (feature-major out [H1, GPC])
            fc1_ps = pp_t1.tile([H1, GPC], F32, tag="t1ps")
            for kt in range(FDP // P):
                ftp = pp_tr.tile([P, GPC], F32, tag="trp")
                nc.tensor.transpose(ftp[:], fus[:, kt * P:(kt + 1) * P],
                                    ident_t[:GPC, :GPC])
                fT = hpool.tile([P, GPC], F32, tag="fT")
                nc.vector.tensor_copy(fT[:], ftp[:])
                nc.tensor.matmul(fc1_ps[:], lhsT=fc1w_t[:, kt, :], rhs=fT[:],
                                 start=(kt == 0), stop=(kt == FDP // P - 1),
                                 skip_group_check=True)

            if PH < 7:
                raise _SkipRest()

            def bn_relu(x_ps, Hdim, g_t, b_t, bni_d, bno_d, tagp):
                xsb = hd.tile([Hdim, GPC], F32, name=f"xsb{tagp}")
                nc.vector.tensor_copy(xsb[:], x_ps[:])
                sums = hd.tile([Hdim, 1], F32, name=f"sums{tagp}")
                nc.vector.reduce_sum(out=sums[:], in_=xsb[:], axis=mybir.AxisListType.X)
                sqj = hd.tile([Hdim, GPC], F32, name=f"sqj{tagp}")
                sumsq = hd.tile([Hdim, 1], F32, name=f"sumsq{tagp}")
                nc.vector.tensor_tensor(out=sqj[:], in0=xsb[:], in1=xsb[:], op=MUL)
                nc.vector.reduce_sum(out=sumsq[:], in_=sqj[:], axis=mybir.AxisListType.X)
                stat = hd.tile([Hdim, 2], F32, name=f"stat{tagp}")
                nc.vector.tensor_copy(stat[:, 0:1], sums[:])
                nc.vector.tensor_copy(stat[:, 1:2], sumsq[:])
                nc.sync.dma_start(bni_d[:], stat[:])
                nc.gpsimd.collective_compute(
                    "AllReduce", ADD, replica_groups=groups,
                    ins=[bni_d[:].opt()], outs=[bno_d[:].opt()])
                statg = hd.tile([Hdim, 2], F32, name=f"statg{tagp}")
                nc.sync.dma_start(statg[:], bno_d[:])
                mean = hd.tile([Hdim, 1], F32, name=f"mean{tagp}")
                nc.vector.tensor_scalar(out=mean[:], in0=statg[:, 0:1],
                                        scalar1=1.0 / B, scalar2=None, op0=MUL)
                var = hd.tile([Hdim, 1], F32, name=f"var{tagp}")
                nc.vector.tensor_scalar(out=var[:], in0=statg[:, 1:2],
                                        scalar1=1.0 / B, scalar2=None, op0=MUL)
                msq = hd.tile([Hdim, 1], F32, name=f"msq{tagp}")
                nc.vector.tensor_tensor(out=msq[:], in0=mean[:], in1=mean[:], op=MUL)
                nc.vector.tensor_tensor(out=var[:], in0=var[:], in1=msq[:], op=SUB)
                nc.vector.tensor_scalar(out=var[:], in0=var[:], scalar1=BN_EPS,
                                        scalar2=None, op0=ADD)
                sd = hd.tile([Hdim, 1], F32, name=f"sd{tagp}")
                nc.scalar.activation(out=sd[:], in_=var[:], func=AF.Sqrt, bias=zcol[:Hdim, :1])
                rsd = hd.tile([Hdim, 1], F32, name=f"rsd{tagp}")
                nc.vector.reciprocal(rsd[:], sd[:])
                scl = hd.tile([Hdim, 1], F32, name=f"scl{tagp}")
                nc.vector.tensor_tensor(out=scl[:], in0=rsd[:], in1=g_t[:], op=MUL)
                tb = hd.tile([Hdim, 1], F32, name=f"tb{tagp}")
                nc.vector.tensor_tensor(out=tb[:], in0=mean[:], in1=scl[:], op=MUL)
                nc.vector.tensor_scalar(out=tb[:], in0=tb[:], scalar1=-1.0,
                                        scalar2=None, op0=MUL)
                nc.vector.tensor_tensor(out=tb[:], in0=tb[:], in1=b_t[:], op=ADD)
                o = hd.tile([Hdim, GPC], F32, name=f"bno{tagp}")
                nc.scalar.activation(out=o[:], in_=xsb[:], func=AF.Relu,
                                     bias=tb[:, 0:1], scale=scl[:, 0:1])
                return o

            bn1o_t = bn_relu(fc1_ps, H1, bn1g_t, bn1b_t, bn1i_d, bn1o_d, "1")
            if PH < 8:
                raise _SkipRest()
            fc2_ps = pp_t1.tile([H2, GPC], F32, tag="t1ps")
            nc.tensor.matmul(fc2_ps[:], lhsT=fc2w_t[:], rhs=bn1o_t[:], start=True, stop=True)
            bn2o_t = bn_relu(fc2_ps, H2, bn2g_t, bn2b_t, bn2i_d, bn2o_d, "2")
            fc3_ps = pp_t1.tile([1, GPC], F32, tag="t1ps")
            nc.tensor.matmul(fc3_ps[:], lhsT=fc3w_t[:], rhs=bn2o_t[:], start=True, stop=True)
            outsb = hd.tile([1, GPC], F32)
            nc.vector.tensor_scalar(out=outsb[:], in0=fc3_ps[:],
                                    scalar1=fc3b_t[0:1, 0:1], scalar2=None, op0=ADD)
            nc.sync.dma_start(out_d[:], outsb[:])
            nc.leave_named_scope("ph6_head", _sc6[0], False)
          except _SkipRest:
            pass

    nc.compile()
    return nc


# ----------------------------------------------------------------------------
# Entry point
# ----------------------------------------------------------------------------

REAL_DIMS = dict(N=50000, E=800000, B=512, DIM_IN=128, GC1=100, DG=20,
                 D2=200, DH=64, H1=128, H2=32)
_CACHE = {}


def run(inputs, nc_cores=8, dims=None, trace=False):
    dims = dims or REAL_DIMS
    meta, per_core = plan(inputs, nc_cores, dims)
    key = repr(sorted(meta.items()))
    if key not in _CACHE:
        _CACHE[key] = build(meta)
    prog = _CACHE[key]
    from concourse.bass_utils import run_bass_kernel_spmd
    res = run_bass_kernel_spmd(prog, per_core, list(range(nc_cores)), trace=trace)
    outs = [np.asarray(res.results[c]["out"]).reshape(-1) for c in range(nc_cores)]
    y = np.concatenate(outs).astype(np.float32)[:, None]
    return y, res


def kernel(**inputs):
    y, _ = run(inputs, nc_cores=8, dims=REAL_DIMS, trace=False)
    return y
